# revision 1
# baseline (speedup 1.0000x reference)
"""Trainium2 Bass kernel for a GNN message-passing layer (BoundaryConvLayer).

Computation (reference, per node i over D=128 channels):
    rate  = softplus(x @ W_rate) + EPS
    gamma = x @ W_rob + b_rob
    h     = x @ W_fc + b_fc
    agg   = segment_sum(h[row] + h[col], row)
    y     = LayerNorm((rate*agg + gamma) / (1 + rate*deg + EPS)) * ln_gamma + ln_beta

Distribution: nodes sharded across 8 cores by contiguous row blocks; edges
partitioned by destination row so the segment sum is local.  Every core
computes the full (bias-free) GEMM g = x @ W_fc and stores it in its own DRAM
copy as the gather table; per-edge h[col] rows are then fetched locally with
the batched DMAGatherAnt instruction.

Key identities used:
    agg[i] = cnt[i]*g[i] + sum_{e:row=i} g[col_e] + 2*cnt[i]*b_fc
  where cnt = in-edge count (h = g + b_fc).  The neighbor sum is a one-hot
  "selection matrix" matmul on the PE over gathered edge rows; the cnt[i]*g[i]
  self term is a diag(cnt) matmul; the bias term is a K=1 matmul.

Gather layout: dma_gather indices are int16, so the g table is split into
NCHK chunks of CPAD (< 32768) rows; nodes are remapped so each chunk holds
RPC real rows followed by zero rows (zero because the padded x columns are
zero), which serve as padding targets for unused grid slots.  Per node tile
and chunk there are Cq 128-slot groups; each (chunk, tile-group) pair is one
dense dma_gather instruction.
"""

import numpy as np
import ml_dtypes
from contextlib import ExitStack
from dataclasses import dataclass

import concourse.bass as bass
import concourse.tile as tile
from concourse import bacc, mybir
from concourse.bass_utils import run_bass_kernel_spmd

# The stock ACT-table chooser greedily picks the first set containing each
# function, which for {Exp, Ln, Copy, Square} alternates between two sets and
# reloads the table ~150x per run (~1.3us each).  Restrict it to the one set
# that contains all four so a single load suffices.
_ACT_KEEP = "natural_log_exp_and_others"
if not getattr(bacc, "_act_tables_patched", False):
    _orig_get_tables = bacc.get_activation_tables

    def _patched_get_tables(arch):
        t = _orig_get_tables(arch)
        if _ACT_KEEP in t:
            t = {k: (v if k == _ACT_KEEP else set()) for k, v in t.items()}
        return t

    bacc.get_activation_tables = _patched_get_tables
    bacc._act_tables_patched = True

BF16 = ml_dtypes.bfloat16
EPS = 1e-4
LN_EPS = 1e-5
P = 128
D = 128


@dataclass
class Cfg:
    N: int            # total nodes
    E: int            # total edges
    NC: int           # cores
    NCHK: int = 4     # gather table chunks (int16 range)
    Cq: int = 0       # 128-slot groups per (tile, chunk); set by prep
    ln_trivial: bool = False

    @property
    def NLOC(self):
        return self.N // self.NC

    @property
    def T(self):
        return (self.NLOC + P - 1) // P

    @property
    def TLP(self):
        return self.T * P

    @property
    def RPC(self):    # real nodes per chunk
        return self.N // self.NCHK

    @property
    def CPAD(self):   # padded chunk rows (>=64 zero rows, 128-aligned)
        return ((self.RPC + 64 + P - 1) // P) * P

    @property
    def NPAD(self):   # g table rows
        return self.NCHK * self.CPAD

    @property
    def G(self):      # tiles per gather group
        for g in (7, 14, 4, 2, 1):
            if self.T % g == 0:
                return g
        return 1


def prep(x, edge_index, degree, W_fc, b_fc, W_rate, W_rob, b_rob, ln_gamma, ln_beta,
         cfg: Cfg):
    """Host-side preprocessing: shard + build per-core gather/selection tables."""
    N, NC, NCHK = cfg.N, cfg.NC, cfg.NCHK
    NLOC, T, TLP = cfg.NLOC, cfg.T, cfg.TLP
    RPC, CPAD, NPAD = cfg.RPC, cfg.CPAD, cfg.NPAD
    assert N % NCHK == 0 and NC % NCHK == 0 and CPAD <= 32767
    CPC = NC // NCHK  # cores per chunk
    # last core of each chunk must fit its padded tile range inside the chunk
    assert (CPC - 1) * NLOC + TLP <= CPAD

    x = np.asarray(x, np.float32)
    edge_index = np.asarray(edge_index, np.int64)
    degree = np.asarray(degree)
    row, col = edge_index[0], edge_index[1]

    xT = np.zeros((P, NPAD), BF16)
    xTf = x.T.astype(BF16)
    for q in range(NCHK):
        xT[:, q * CPAD:q * CPAD + RPC] = xTf[:, q * RPC:(q + 1) * RPC]

    w_fc = np.ascontiguousarray(W_fc, dtype=np.float32).astype(BF16)
    w_rt = np.ascontiguousarray(W_rate, dtype=np.float32).astype(BF16)
    w_rb = np.ascontiguousarray(W_rob, dtype=np.float32).astype(BF16)
    brow = np.zeros((1, 2 * D), np.float32)
    brow[0, :D] = b_fc
    brow[0, D:] = b_rob
    onesrow = np.ones((1, D), np.float32)
    ident = np.eye(P, dtype=BF16)

    cfg.ln_trivial = bool(np.all(np.asarray(ln_gamma) == 1.0)
                          and np.all(np.asarray(ln_beta) == 0.0))
    lnab = np.zeros((P, 2 * D), np.float32)
    lnab[:, :D] = np.asarray(ln_gamma, np.float32)[None, :]
    lnab[:, D:] = np.asarray(ln_beta, np.float32)[None, :]

    core_of = row // NLOC
    chunk_of_col = col // RPC

    # pass 1: per-core, per-tile, per-chunk edge counts fix the global Cq
    percore = []
    maxslots = 0
    for r in range(NC):
        m = core_of == r
        rl = row[m] - r * NLOC
        ce = col[m]
        cq = chunk_of_col[m]
        cnt = np.bincount(rl, minlength=TLP)
        tq = (rl // P) * NCHK + cq
        cnt_tq = np.bincount(tq, minlength=T * NCHK).reshape(T, NCHK)
        maxslots = max(maxslots, int(cnt_tq.max()))
        percore.append((rl, ce, cq, cnt, cnt_tq))
    Cq = max(1, -(-maxslots // P))
    cfg.Cq = Cq
    NG = T // cfg.G
    IPG = cfg.G * Cq * P       # idxs per (chunk, group) instruction

    in_maps = []
    for r in range(NC):
        rl, ce, cq, cnt, cnt_tq = percore[r]
        # order edges by (tile, chunk, row) to fill grids densely
        order = np.lexsort((rl, cq, rl // P))
        rl_s, ce_s, cq_s = rl[order], ce[order], cq[order]
        t_s = rl_s // P
        tq_s = t_s * NCHK + cq_s
        run_start = np.zeros(T * NCHK + 1, np.int64)
        np.cumsum(cnt_tq.reshape(-1), out=run_start[1:])
        pos = np.arange(len(rl_s)) - run_start[tq_s]
        # idx-stream position within instruction (q, gg):
        #   msg layout [P, tile-in-group, q, c, D]; stream for chunk q covers
        #   slots [tl, c, p] in that order -> i = tl*(Cq*128) + c*128 + p
        tl_s = t_s % cfg.G
        gg_s = t_s // cfg.G
        ipos = tl_s * (Cq * P) + pos
        idx16 = np.full((NCHK, NG, IPG), RPC, np.int16)  # pad -> zero row
        idx16[cq_s, gg_s, ipos] = (ce_s % RPC).astype(np.int16)
        # wrap each stream: idx i -> [i%16, i//16], replicate to 128 partitions
        idxw = idx16.reshape(NCHK, NG, IPG // 16, 16).transpose(0, 1, 3, 2)
        idxw = np.ascontiguousarray(idxw)
        idxw = np.tile(idxw, (1, 1, 8, 1))           # [NCHK, NG, 128, IPG//16]
        idx_sb = np.ascontiguousarray(
            idxw.transpose(2, 0, 1, 3)).reshape(P, NCHK * NG * (IPG // 16))

        # rowsr: rebased row (node % 128) per slot, -1 for pads
        rowsr = np.full((P, T * NCHK * Cq), -1.0, BF16)
        slot_col = t_s * (NCHK * Cq) + cq_s * Cq + pos // P
        rowsr[pos % P, slot_col] = (rl_s % P).astype(BF16)

        iotab = np.broadcast_to(
            np.tile(np.arange(P, dtype=BF16)[None, :], (1, NCHK * Cq)),
            (P, NCHK * Cq * P)).copy()

        cnt2 = (2.0 * cnt).astype(np.float32)[None, :]
        cntb = cnt.astype(np.float32).reshape(T, P).T.copy()
        degl = np.zeros(TLP, np.float32)
        degl[:NLOC] = degree[r * NLOC:(r + 1) * NLOC].astype(np.float32)
        degf = degl.reshape(T, P).T.copy()
        g0 = (r * NLOC // RPC) * CPAD + (r * NLOC % RPC)
        xTloc = np.ascontiguousarray(xT[:, g0:g0 + TLP])

        in_maps.append({
            "xT": xT, "xTloc": xTloc,
            "Wfc": w_fc, "Wrt": w_rt, "Wrb": w_rb,
            "brow": brow, "onesrow": onesrow, "ident": ident, "lnab": lnab,
            "iotab": iotab, "rowsr": rowsr, "idxs": idx_sb,
            "cnt2": cnt2, "cntb": cntb, "degf": degf,
        })
    return in_maps


def build(cfg: Cfg):
    """Build the SPMD Bass program (identical on every core)."""
    NC, T, TLP, NPAD = cfg.NC, cfg.T, cfg.TLP, cfg.NPAD
    NCHK, Cq, CPAD = cfg.NCHK, cfg.Cq, cfg.CPAD
    G = cfg.G
    NG = T // G
    IPG = G * Cq * P
    SELW = NCHK * Cq * P       # sel width per tile
    bf = mybir.dt.bfloat16
    f32 = mybir.dt.float32
    i16 = mybir.dt.int16

    nc = bacc.Bacc("TRN2", target_bir_lowering=False, debug=False, num_devices=NC,
                   num_swdge_queues=4)
    cs = nc.alloc_sbuf_tensor(f"const-float32-{LN_EPS}", [P, 1], f32)
    nc.gpsimd.memset(cs.ap(), LN_EPS)
    nc.const_aps.aps[(f32, LN_EPS)] = cs.ap()
    nc.all_engine_barrier()

    d_xT = nc.dram_tensor("xT", [P, NPAD], bf, kind="ExternalInput").ap()
    d_xTloc = nc.dram_tensor("xTloc", [P, TLP], bf, kind="ExternalInput").ap()
    d_wfc = nc.dram_tensor("Wfc", [P, D], bf, kind="ExternalInput").ap()
    d_wrt = nc.dram_tensor("Wrt", [P, D], bf, kind="ExternalInput").ap()
    d_wrb = nc.dram_tensor("Wrb", [P, D], bf, kind="ExternalInput").ap()
    d_brow = nc.dram_tensor("brow", [1, 2 * D], f32, kind="ExternalInput").ap()
    d_ones = nc.dram_tensor("onesrow", [1, D], f32, kind="ExternalInput").ap()
    d_ident = nc.dram_tensor("ident", [P, P], bf, kind="ExternalInput").ap()
    d_lnab = nc.dram_tensor("lnab", [P, 2 * D], f32, kind="ExternalInput").ap()
    d_iota = nc.dram_tensor("iotab", [P, SELW], bf, kind="ExternalInput").ap()
    d_rowsr = nc.dram_tensor("rowsr", [P, T * NCHK * Cq], bf,
                             kind="ExternalInput").ap()
    d_idxs = nc.dram_tensor("idxs", [P, NCHK * NG * (IPG // 16)], i16,
                            kind="ExternalInput").ap()
    d_cnt2 = nc.dram_tensor("cnt2", [1, TLP], f32, kind="ExternalInput").ap()
    d_cntb = nc.dram_tensor("cntb", [P, T], f32, kind="ExternalInput").ap()
    d_degf = nc.dram_tensor("degf", [P, T], f32, kind="ExternalInput").ap()
    # one g-table tensor per chunk so chunk-q gathers depend only on chunk-q
    # phase-1 writes (phase-1/phase-3 overlap via per-queue FIFOs)
    d_gq = [nc.dram_tensor(f"gtab{q}", [CPAD, D], bf, kind="Internal").ap()
            for q in range(NCHK)]
    d_y = nc.dram_tensor("y", [TLP, D], f32, kind="ExternalOutput").ap()

    with tile.TileContext(nc) as tc, ExitStack() as ctx:
        from concourse import library_config
        nc.gpsimd.load_library(library_config.mlp)
        consts = ctx.enter_context(tc.tile_pool(name="consts", bufs=1))
        wfc = consts.tile([P, D], bf)
        nc.sync.dma_start(wfc[:], d_wfc[:])

        # ---------------- phase 1: g = x @ W_fc for all nodes ----------------
        CHUNK = 8192
        GRP = 512
        with tc.tile_pool(name="p1x", bufs=2) as p1x, \
             tc.tile_pool(name="p1ps", bufs=2, space="PSUM") as p1ps, \
             tc.tile_pool(name="p1st", bufs=3) as p1st:
            for q in range(NCHK):
                for c0 in range(0, CPAD, CHUNK):
                    cw = min(CHUNK, CPAD - c0)
                    xc = p1x.tile([P, CHUNK], bf, tag="xc", name="xc")
                    nc.sync.dma_start(xc[:, :cw],
                                      d_xT[:, q * CPAD + c0:q * CPAD + c0 + cw])
                    for g0 in range(0, cw, GRP):
                        gw = min(GRP, cw - g0)
                        gps = p1ps.tile([P, GRP], f32, space="PSUM", tag="gps",
                                        name="gps")
                        for j in range(0, gw, P):
                            nc.tensor.matmul(
                                out=gps[:, j:j + P],
                                lhsT=xc[:, g0 + j:g0 + j + P],
                                rhs=wfc[:],
                                start=True, stop=True,
                            )
                        gst = p1st.tile([P, GRP], bf, tag="gst", name="gst")
                        nc.scalar.copy(gst[:, :gw], gps[:, :gw])
                        dst = d_gq[q][c0 + g0:c0 + g0 + gw, :].rearrange(
                            "(t p) d -> p t d", p=P)
                        nc.sync.dma_start(dst, gst[:, :gw].rearrange(
                            "p (t d) -> p t d", d=D))

        # ---------------- phase 3: message passing + elementwise -------------
        wrt = consts.tile([P, D], bf)
        nc.sync.dma_start(wrt[:], d_wrt[:])
        wrb = consts.tile([P, D], bf)
        nc.sync.dma_start(wrb[:], d_wrb[:])
        brow = consts.tile([1, 2 * D], f32)
        nc.sync.dma_start(brow[:], d_brow[:])
        onesr = consts.tile([1, D], f32)
        nc.sync.dma_start(onesr[:], d_ones[:])
        ident = consts.tile([P, P], bf)
        nc.sync.dma_start(ident[:], d_ident[:])
        iota = consts.tile([P, SELW], bf)
        nc.sync.dma_start(iota[:], d_iota[:])
        rowsr = consts.tile([P, T * NCHK * Cq], bf)
        nc.sync.dma_start(rowsr[:], d_rowsr[:])
        idxs = consts.tile([P, NCHK * NG * (IPG // 16)], i16)
        nc.sync.dma_start(idxs[:], d_idxs[:])
        cnt2 = consts.tile([1, TLP], f32)
        nc.sync.dma_start(cnt2[:], d_cnt2[:])
        cntb = consts.tile([P, T], f32)
        nc.sync.dma_start(cntb[:], d_cntb[:])
        degf = consts.tile([P, T], f32)
        nc.sync.dma_start(degf[:], d_degf[:])
        xloc = consts.tile([P, TLP], bf)
        nc.sync.dma_start(xloc[:], d_xTloc[:])
        lnab = None
        if not cfg.ln_trivial:
            lnab = consts.tile([P, 2 * D], f32)
            nc.sync.dma_start(lnab[:], d_lnab[:])

        msgp = ctx.enter_context(tc.tile_pool(name="msgp", bufs=2))
        selp = ctx.enter_context(tc.tile_pool(name="selp", bufs=3))
        aggps = ctx.enter_context(tc.tile_pool(name="aggps", bufs=2, space="PSUM"))
        ratps = ctx.enter_context(tc.tile_pool(name="ratps", bufs=2, space="PSUM"))
        gamps = ctx.enter_context(tc.tile_pool(name="gamps", bufs=2, space="PSUM"))
        glps = ctx.enter_context(tc.tile_pool(name="glps", bufs=2, space="PSUM"))
        eltp = ctx.enter_context(tc.tile_pool(name="eltp", bufs=2))
        smallp = ctx.enter_context(tc.tile_pool(name="smallp", bufs=2))
        B = 4

        def eltwise(bt, tiles):
            nb = len(tiles)
            rate4, agg4, gam4 = bt
            r3 = rate4[:, :nb, :]
            a3 = agg4[:, :nb, :]
            g3 = gam4[:, :nb, :]
            num = eltp.tile([P, B, D], f32, tag="num", name="num")[:, :nb, :]
            den = eltp.tile([P, B, D], f32, tag="den", name="den")[:, :nb, :]
            y0 = eltp.tile([P, B, D], f32, tag="y0", name="y0")[:, :nb, :]
            sq = eltp.tile([P, B, D], f32, tag="sq", name="sq")[:, :nb, :]
            yf = eltp.tile([P, B, D], f32, tag="yf", name="yf")
            st = smallp.tile([P, 8 * B], f32, tag="st", name="st")
            s1 = st[:, 0:nb]
            s2 = st[:, B:B + nb]
            mean = st[:, 2 * B:2 * B + nb]
            msq = st[:, 3 * B:3 * B + nb]
            var = st[:, 4 * B:4 * B + nb]
            rstd = st[:, 5 * B:5 * B + nb]

            nc.vector.scalar_tensor_tensor(
                out=num, in0=r3, scalar=EPS, in1=a3,
                op0=mybir.AluOpType.add, op1=mybir.AluOpType.mult)
            nc.vector.tensor_add(out=num, in0=num, in1=g3)
            t0g = tiles[0]
            degb = degf[:, t0g:t0g + nb][:, :, None].to_broadcast([P, nb, D])
            nc.vector.scalar_tensor_tensor(
                out=den, in0=r3, scalar=EPS, in1=degb,
                op0=mybir.AluOpType.add, op1=mybir.AluOpType.mult)
            nc.vector.tensor_scalar_add(out=den, in0=den, scalar1=1.0 + EPS)
            nc.vector.reciprocal(out=den, in_=den)
            nc.vector.tensor_mul(out=y0, in0=num, in1=den)
            nc.scalar.square(sq, y0)
            nc.vector.tensor_reduce(out=s1, in_=y0, axis=mybir.AxisListType.X,
                                    op=mybir.AluOpType.add)
            nc.vector.tensor_reduce(out=s2, in_=sq, axis=mybir.AxisListType.X,
                                    op=mybir.AluOpType.add)
            nc.vector.tensor_scalar_mul(out=mean, in0=s1, scalar1=1.0 / D)
            nc.vector.tensor_scalar_mul(out=msq, in0=s2, scalar1=1.0 / D)
            nc.vector.tensor_tensor(out=var, in0=mean, in1=mean,
                                    op=mybir.AluOpType.mult)
            nc.vector.tensor_sub(out=var, in0=msq, in1=var)
            # rstd = (var+eps)^-0.5 = exp(-0.5*ln(var+eps)); exp/ln share one
            # ACT table with the softplus pieces, so no table reloads
            nc.scalar.activation(out=var, in_=var,
                                 func=mybir.ActivationFunctionType.Ln,
                                 bias=LN_EPS)
            nc.scalar.activation(out=rstd, in_=var,
                                 func=mybir.ActivationFunctionType.Exp,
                                 scale=-0.5)
            meanb = mean[:, :, None].to_broadcast([P, nb, D])
            rstdb = rstd[:, :, None].to_broadcast([P, nb, D])
            yf3 = yf[:, :nb, :]
            nc.vector.tensor_sub(out=yf3, in0=y0, in1=meanb)
            nc.vector.tensor_mul(out=yf3, in0=yf3, in1=rstdb)
            if lnab is not None:
                lg = lnab[:, 0:D][:, None, :].to_broadcast([P, nb, D])
                lb = lnab[:, D:2 * D][:, None, :].to_broadcast([P, nb, D])
                nc.vector.tensor_mul(out=yf3, in0=yf3, in1=lg)
                nc.vector.tensor_add(out=yf3, in0=yf3, in1=lb)
            n0 = tiles[0] * P
            nw = nb * P
            dst = d_y[n0:n0 + nw, :].rearrange("(t p) d -> p t d", p=P)
            nc.sync.dma_start(dst, yf[:, :nb, :])

        bt = None
        for gg in range(NG):
            tg0 = gg * G
            # msg layout: [P, q, tile-in-group, c, D] -- q outermost so each
            # chunk's gather writes one contiguous [P, G*Cq, D] section
            msg = msgp.tile([P, NCHK * G * Cq * D], bf, tag="msg", name="msg")
            for q in range(NCHK):
                icol = (q * NG + gg) * (IPG // 16)
                sec = msg[:, q * G * Cq * D:(q + 1) * G * Cq * D]
                nc.gpsimd.dma_gather(
                    out_ap=sec.rearrange("p (s d) -> p s d", d=D),
                    in_ap=d_gq[q][:],
                    idxs_ap=idxs[:, icol:icol + IPG // 16],
                    num_idxs=IPG,
                    num_idxs_reg=IPG,
                    elem_size=D,
                    single_packet=False,
                    queue_num=q % 4,
                )
            for tl in range(G):
                t = tg0 + tl
                j = t % B
                if j == 0:
                    bt = (eltp.tile([P, B, D], f32, tag="rate4", name="rate4"),
                          eltp.tile([P, B, D], f32, tag="agg4", name="agg4"),
                          eltp.tile([P, B, D], f32, tag="gam4", name="gam4"))
                sel = selp.tile([P, SELW], bf, tag="sel", name="sel")
                rb = rowsr[:, t * NCHK * Cq:(t + 1) * NCHK * Cq][:, :, None] \
                    .to_broadcast([P, NCHK * Cq, P])
                nc.vector.tensor_tensor(
                    out=sel.rearrange("p (c m) -> p c m", c=NCHK * Cq), in0=rb,
                    in1=iota.rearrange("p (c m) -> p c m", c=NCHK * Cq),
                    op=mybir.AluOpType.is_equal)
                diag = selp.tile([P, P], bf, tag="diag", name="diag")
                nc.vector.tensor_scalar_mul(out=diag[:], in0=ident[:],
                                            scalar1=cntb[:, t:t + 1])
                # local g rows for the self term: recompute on the PE
                glp = glps.tile([P, D], f32, space="PSUM", tag="glp", name="glp")
                nc.tensor.matmul(out=glp[:], lhsT=xloc[:, t * P:(t + 1) * P],
                                 rhs=wfc[:], start=True, stop=True)
                gl = selp.tile([P, D], bf, tag="gl", name="gl")
                nc.scalar.copy(gl[:], glp[:])
                aps = aggps.tile([P, D], f32, space="PSUM", tag="aps", name="aps")
                nc.tensor.matmul(out=aps[:], lhsT=diag[:], rhs=gl[:],
                                 start=True, stop=False)
                for q in range(NCHK):
                    for c in range(Cq):
                        cc = q * Cq + c
                        moff = ((q * G + tl) * Cq + c) * D
                        nc.tensor.matmul(
                            out=aps[:], lhsT=sel[:, cc * P:(cc + 1) * P],
                            rhs=msg[:, moff:moff + D],
                            start=False, stop=False)
                nc.tensor.matmul(out=aps[:], lhsT=cnt2[0:1, t * P:(t + 1) * P],
                                 rhs=brow[0:1, 0:D], start=False, stop=True)
                rps = ratps.tile([P, D], f32, space="PSUM", tag="rps", name="rps")
                nc.tensor.matmul(out=rps[:], lhsT=xloc[:, t * P:(t + 1) * P],
                                 rhs=wrt[:], start=True, stop=True)
                gps = gamps.tile([P, D], f32, space="PSUM", tag="gps3",
                                 name="gps3")
                nc.tensor.matmul(out=gps[:], lhsT=xloc[:, t * P:(t + 1) * P],
                                 rhs=wrb[:], start=True, stop=False)
                nc.tensor.matmul(out=gps[:], lhsT=onesr[0:1, :],
                                 rhs=brow[0:1, D:2 * D], start=False, stop=True)
                # softplus(z) = ln(exp(z)+1): one ACT table (exp/ln) throughout
                spt = selp.tile([P, D], f32, tag="spt", name="spt")
                nc.scalar.activation(out=spt[:], in_=rps[:],
                                     func=mybir.ActivationFunctionType.Exp)
                nc.scalar.activation(out=bt[0][:, j, :], in_=spt[:],
                                     func=mybir.ActivationFunctionType.Ln,
                                     bias=1.0)
                nc.scalar.copy(bt[1][:, j, :], aps[:])
                nc.scalar.copy(bt[2][:, j, :], gps[:])
                if j == B - 1 or t == T - 1:
                    eltwise(bt, list(range(t - j, t + 1)))

    nc.compile()
    return nc


def run(inputs, cfg: Cfg, core_ids=None):
    in_maps = prep(**inputs, cfg=cfg)
    nc = build(cfg)
    res = run_bass_kernel_spmd(nc, in_maps, core_ids=core_ids or list(range(cfg.NC)))
    ys = [res.results[r]["y"][:cfg.NLOC] for r in range(cfg.NC)]
    return np.concatenate(ys, axis=0)


def kernel(**inputs):
    cfg = Cfg(N=100_000, E=800_000, NC=8)
    return run(inputs, cfg)



# revision 9
# speedup vs baseline: 2.2073x; 2.2073x over previous
"""Trainium2 Bass kernel for a GNN message-passing layer (BoundaryConvLayer).

Computation (reference, per node i over D=128 channels):
    rate  = softplus(x @ W_rate) + EPS
    gamma = x @ W_rob + b_rob
    h     = x @ W_fc + b_fc
    agg   = segment_sum(h[row] + h[col], row)
    y     = LayerNorm((rate*agg + gamma) / (1 + rate*deg + EPS)) * ln_gamma + ln_beta

Distribution: nodes sharded across 8 cores by contiguous row blocks; edges
partitioned by destination row so the segment sum is local to each core.

Key identity (g = x @ W_fc, cnt = in-edge count):
    agg[i] = (sum_{e:row=i} x[col_e]) @ W_fc + cnt[i]*g[i] + 2*cnt[i]*b_fc
The inner segment sum runs in INPUT space: the host stages the per-edge
source rows x[col_e] (pure indexing, no host FLOPs) grouped by destination
tile, and the PE reduces each 128-slot group with a one-hot "selection
matrix" matmul accumulated in PSUM.  This removes the device-side table
gather (the former software-DGE bottleneck) and the redundant full-N GEMM:
all remaining DMA is large contiguous hardware-queue traffic.

Per 128-node tile t (S slot groups each):
    sumxT [x,dst] = sum_s  xe_grp[slot,x].T-matmul  sel[slot,dst]   (PE)
    agg   [dst,d] = sumxT.T @ Wfc + diag(cnt) @ g_loc + cnt2 (x) b_fc
    xw3   [n,384] = xlocT.T @ [Wfc|Wrate|Wrob] (+ b_rob in cols 256:384)
    eltwise + LayerNorm on DVE/ACT/Pool; 1/den and rsqrt via exp/ln so a
    single ACT table load suffices.
"""

import numpy as np
import ml_dtypes
from contextlib import ExitStack
from dataclasses import dataclass

import concourse.bass as bass
import concourse.tile as tile
from concourse import bacc, mybir
from concourse.bass_utils import run_bass_kernel_spmd

# The stock ACT-table chooser greedily picks the first set containing each
# function, which for {Exp, Ln, Copy} can alternate between two sets and
# reload the table per use (~1.3us each).  Restrict it to the one set that
# contains all of them so a single load suffices.
_ACT_KEEP = "natural_log_exp_and_others"
if not getattr(bacc, "_act_tables_patched", False):
    _orig_get_tables = bacc.get_activation_tables

    def _patched_get_tables(arch):
        t = _orig_get_tables(arch)
        if _ACT_KEEP in t:
            t = {k: (v if k == _ACT_KEEP else set()) for k, v in t.items()}
        return t

    bacc.get_activation_tables = _patched_get_tables
    bacc._act_tables_patched = True

BF16 = ml_dtypes.bfloat16
EPS = 1e-4
LN_EPS = 1e-5
P = 128
D = 128

# HW-compat switches (CoreSim accepts all; flipped off while bisecting a
# hardware INTERNAL failure)
USE_TTR = False           # fused tensor_tensor_reduce for y0/sq row sums
USE_ACT_APSCALE = False   # activation Ln with per-partition scale/bias APs
USE_STT_AP = False        # scalar_tensor_tensor with per-partition scalar AP


@dataclass
class Cfg:
    N: int            # total nodes
    E: int            # total edges
    NC: int           # cores
    S: int = 0        # slot groups per tile (set by prep)
    SG: int = 7       # tiles per super-group (pipelining granule)
    ln_trivial: bool = False

    @property
    def NLOC(self):
        return self.N // self.NC

    @property
    def T(self):
        return (self.NLOC + P - 1) // P

    @property
    def TLP(self):
        return self.T * P

    @property
    def NSG(self):
        assert self.T % self.SG == 0
        return self.T // self.SG


def prep(x, edge_index, degree, W_fc, b_fc, W_rate, W_rob, b_rob, ln_gamma, ln_beta,
         cfg: Cfg):
    """Host-side preprocessing: shard + stage per-edge source rows by dst tile."""
    N, NC = cfg.N, cfg.NC
    NLOC, T, TLP = cfg.NLOC, cfg.T, cfg.TLP

    x = np.asarray(x, np.float32)
    edge_index = np.asarray(edge_index, np.int64)
    degree = np.asarray(degree)
    row, col = edge_index[0], edge_index[1]

    xbf = x.astype(BF16)
    xbf_ext = np.concatenate([xbf, np.zeros((1, D), BF16)], axis=0)  # pad row

    w3 = np.zeros((P, 3 * D), BF16)
    w3[:, 0:D] = np.asarray(W_fc, np.float32).astype(BF16)
    w3[:, D:2 * D] = np.asarray(W_rate, np.float32).astype(BF16)
    w3[:, 2 * D:3 * D] = np.asarray(W_rob, np.float32).astype(BF16)
    brow3 = np.zeros((1, 3 * D), np.float32)
    brow3[0, 2 * D:3 * D] = np.asarray(b_rob, np.float32)
    bfcrow = np.asarray(b_fc, np.float32).reshape(1, D).copy()
    onesrow = np.ones((1, D), np.float32)
    ident = np.eye(P, dtype=BF16)

    cfg.ln_trivial = bool(np.all(np.asarray(ln_gamma) == 1.0)
                          and np.all(np.asarray(ln_beta) == 0.0))
    lnab = np.zeros((P, 2 * D), np.float32)
    lnab[:, :D] = np.asarray(ln_gamma, np.float32)[None, :]
    lnab[:, D:] = np.asarray(ln_beta, np.float32)[None, :]

    core_of = row // NLOC

    # pass 1: per-core per-tile edge counts fix the global S (slot groups/tile)
    percore = []
    S = 1
    for r in range(NC):
        m = core_of == r
        rl = row[m] - r * NLOC
        ce = col[m]
        cnt = np.bincount(rl, minlength=TLP)
        cnt_t = np.bincount(rl // P, minlength=T)
        S = max(S, -(-int(cnt_t.max()) // P))
        percore.append((rl, ce, cnt, cnt_t))
    cfg.S = S
    SPT = S * P  # slots per tile

    in_maps = []
    for r in range(NC):
        rl, ce, cnt, cnt_t = percore[r]
        # order edges by (tile, col) -> fill each tile's slots densely; the
        # col ordering gives the staging gather some source locality
        order = np.lexsort((ce, rl // P))
        rl_s, ce_s = rl[order], ce[order]
        t_s = rl_s // P
        run_start = np.zeros(T + 1, np.int64)
        np.cumsum(cnt_t, out=run_start[1:])
        pos = np.arange(len(rl_s)) - run_start[t_s]     # slot within tile
        slot = t_s * SPT + pos                           # global slot id

        # staged source rows, SBUF layout [128, T*S*128] bf16:
        # partition = slot % 128, free = (slot//128)*128 + d
        src = np.full(T * SPT, N, np.int64)              # pads -> zero row
        src[slot] = ce_s
        xe_sb = np.ascontiguousarray(
            xbf_ext[src].reshape(T * S, P, D).transpose(1, 0, 2)
        ).reshape(P, T * S * D)

        # rowsr: dst-in-tile per slot, -1 for pads -> zero sel column
        rowsr = np.full((P, T * S), -1.0, BF16)
        rowsr[pos % P, t_s * S + pos // P] = (rl_s % P).astype(BF16)

        iotab = np.broadcast_to(
            np.arange(P, dtype=BF16)[None, None, :], (P, S, P)
        ).reshape(P, SPT).copy()

        cnt2 = (2.0 * cnt).astype(np.float32)[None, :]
        cntb = cnt.astype(np.float32).reshape(T, P).T.copy()
        degl = np.zeros(TLP, np.float32)
        degl[:NLOC] = degree[r * NLOC:(r + 1) * NLOC].astype(np.float32)
        degf = degl.reshape(T, P).T.copy()
        degb2 = (EPS * degf + (1.0 + EPS)).astype(np.float32)

        xTloc = np.zeros((P, TLP), BF16)
        xTloc[:, :NLOC] = xbf[r * NLOC:(r + 1) * NLOC].T

        in_maps.append({
            "xe": xe_sb, "xT": xTloc,
            "w3": w3, "brow3": brow3, "bfcrow": bfcrow,
            "onesrow": onesrow, "ident": ident, "lnab": lnab,
            "iotab": iotab, "rowsr": rowsr,
            "cnt2": cnt2, "cntb": cntb, "degf": degf, "degb2": degb2,
        })
    return in_maps


def build(cfg: Cfg):
    """Build the SPMD Bass program (identical on every core)."""
    NC, T, TLP = cfg.NC, cfg.T, cfg.TLP
    S, SG, NSG = cfg.S, cfg.SG, cfg.NSG
    SPT = S * P
    bf = mybir.dt.bfloat16
    f32 = mybir.dt.float32
    AO = mybir.AluOpType
    AF = mybir.ActivationFunctionType

    nc = bacc.Bacc("TRN2", target_bir_lowering=False, debug=False, num_devices=NC)
    for val in (LN_EPS, 1.0 + EPS):
        cs = nc.alloc_sbuf_tensor(f"const-float32-{val}", [P, 1], f32)
        nc.gpsimd.memset(cs.ap(), val)
        nc.const_aps.aps[(f32, val)] = cs.ap()
    nc.all_engine_barrier()

    d_xe = nc.dram_tensor("xe", [P, T * S * D], bf, kind="ExternalInput").ap()
    d_xT = nc.dram_tensor("xT", [P, TLP], bf, kind="ExternalInput").ap()
    d_w3 = nc.dram_tensor("w3", [P, 3 * D], bf, kind="ExternalInput").ap()
    d_brow3 = nc.dram_tensor("brow3", [1, 3 * D], f32, kind="ExternalInput").ap()
    d_bfc = nc.dram_tensor("bfcrow", [1, D], f32, kind="ExternalInput").ap()
    d_ones = nc.dram_tensor("onesrow", [1, D], f32, kind="ExternalInput").ap()
    d_ident = nc.dram_tensor("ident", [P, P], bf, kind="ExternalInput").ap()
    d_lnab = nc.dram_tensor("lnab", [P, 2 * D], f32, kind="ExternalInput").ap()
    d_iota = nc.dram_tensor("iotab", [P, SPT], bf, kind="ExternalInput").ap()
    d_rowsr = nc.dram_tensor("rowsr", [P, T * S], bf, kind="ExternalInput").ap()
    d_cnt2 = nc.dram_tensor("cnt2", [1, TLP], f32, kind="ExternalInput").ap()
    d_cntb = nc.dram_tensor("cntb", [P, T], f32, kind="ExternalInput").ap()
    d_degf = nc.dram_tensor("degf", [P, T], f32, kind="ExternalInput").ap()
    d_degb2 = nc.dram_tensor("degb2", [P, T], f32, kind="ExternalInput").ap()
    d_y = nc.dram_tensor("y", [TLP, D], bf, kind="ExternalOutput").ap()

    with tile.TileContext(nc) as tc, ExitStack() as ctx:
        from concourse import library_config
        nc.gpsimd.load_library(library_config.standard)
        consts = ctx.enter_context(tc.tile_pool(name="consts", bufs=1))
        w3 = consts.tile([P, 3 * D], bf)
        nc.sync.dma_start(w3[:], d_w3[:])
        xlocT = consts.tile([P, TLP], bf)
        nc.sync.dma_start(xlocT[:], d_xT[:])
        brow3 = consts.tile([1, 3 * D], f32)
        nc.sync.dma_start(brow3[:], d_brow3[:])
        bfcrow = consts.tile([1, D], f32)
        nc.sync.dma_start(bfcrow[:], d_bfc[:])
        onesr = consts.tile([1, D], f32)
        nc.sync.dma_start(onesr[:], d_ones[:])
        ident = consts.tile([P, P], bf)
        nc.sync.dma_start(ident[:], d_ident[:])
        iota = consts.tile([P, SPT], bf)
        nc.sync.dma_start(iota[:], d_iota[:])
        rowsr = consts.tile([P, T * S], bf)
        nc.sync.dma_start(rowsr[:], d_rowsr[:])
        cnt2 = consts.tile([1, TLP], f32)
        nc.sync.dma_start(cnt2[:], d_cnt2[:])
        cntb = consts.tile([P, T], f32)
        nc.sync.dma_start(cntb[:], d_cntb[:])
        degf = consts.tile([P, T], f32)
        nc.sync.dma_start(degf[:], d_degf[:])
        degb2 = consts.tile([P, T], f32)
        nc.sync.dma_start(degb2[:], d_degb2[:])
        lnab = None
        if not cfg.ln_trivial:
            lnab = consts.tile([P, 2 * D], f32)
            nc.sync.dma_start(lnab[:], d_lnab[:])

        xep = ctx.enter_context(tc.tile_pool(name="xep", bufs=3))
        selp = ctx.enter_context(tc.tile_pool(name="selp", bufs=3))
        xw3ps = ctx.enter_context(tc.tile_pool(name="xw3ps", bufs=2, space="PSUM"))
        sxps = ctx.enter_context(tc.tile_pool(name="sxps", bufs=2, space="PSUM"))
        aggps = ctx.enter_context(tc.tile_pool(name="aggps", bufs=2, space="PSUM"))
        sbp = ctx.enter_context(tc.tile_pool(name="sbp", bufs=3))
        y0p = ctx.enter_context(tc.tile_pool(name="y0p", bufs=2))
        ysgp = ctx.enter_context(tc.tile_pool(name="ysgp", bufs=2))
        stp = ctx.enter_context(tc.tile_pool(name="stp", bufs=2))

        for sg in range(NSG):
            t0 = sg * SG
            xe = xep.tile([P, SG * SPT], bf, tag="xe", name="xe")
            nc.sync.dma_start(xe[:], d_xe[:, t0 * SPT:(t0 + SG) * SPT])
            y0sg = y0p.tile([P, SG, D], f32, tag="y0", name="y0")
            yt = ysgp.tile([P, SG, D], bf, tag="yt", name="yt")
            st = stp.tile([P, 8 * SG], f32, tag="st", name="st")
            s1 = st[:, 0 * SG:1 * SG]
            s2 = st[:, 1 * SG:2 * SG]
            mean = st[:, 2 * SG:3 * SG]
            msq = st[:, 3 * SG:4 * SG]
            var = st[:, 4 * SG:5 * SG]
            rstd = st[:, 5 * SG:6 * SG]
            mrs = st[:, 6 * SG:7 * SG]

            for tl in range(SG):
                t = t0 + tl
                # xw3 = x_tile @ [Wfc|Wrate|Wrob] (+ bias row in cols 256:384)
                xw3 = xw3ps.tile([P, 3 * D], f32, space="PSUM", tag="xw3",
                                 name="xw3")
                nc.tensor.matmul(out=xw3[:], lhsT=xlocT[:, t * P:(t + 1) * P],
                                 rhs=w3[:], start=True, stop=False)
                nc.tensor.matmul(out=xw3[:], lhsT=onesr[0:1, :],
                                 rhs=brow3[0:1, :], start=False, stop=True)
                # one-hot slot->dst selection per group (DVE) + diag(cnt) (Pool)
                sel = selp.tile([P, SPT], bf, tag="sel", name="sel")
                rb = rowsr[:, t * S:(t + 1) * S][:, :, None] \
                    .to_broadcast([P, S, P])
                nc.vector.tensor_tensor(
                    out=sel.rearrange("p (s m) -> p s m", s=S), in0=rb,
                    in1=iota.rearrange("p (s m) -> p s m", s=S),
                    op=AO.is_equal)
                diag = selp.tile([P, P], bf, tag="diag", name="diag")
                nc.gpsimd.tensor_tensor(
                    out=diag[:], in0=ident[:],
                    in1=cntb[:, t:t + 1].to_broadcast([P, P]), op=AO.mult)
                # sumxT[x, dst] accumulated over the tile's S slot groups
                sx = sxps.tile([P, D], f32, space="PSUM", tag="sx", name="sx")
                for s in range(S):
                    g0 = (tl * S + s) * D
                    nc.tensor.matmul(out=sx[:], lhsT=xe[:, g0:g0 + D],
                                     rhs=sel[:, s * P:(s + 1) * P],
                                     start=(s == 0), stop=(s == S - 1))
                sxT = sbp.tile([P, D], bf, tag="sxT", name="sxT")
                nc.scalar.copy(sxT[:], sx[:])
                gl = sbp.tile([P, D], bf, tag="gl", name="gl")
                nc.scalar.copy(gl[:], xw3[:, 0:D])
                # agg = sumxT.T @ Wfc + diag(cnt) @ g_loc + 2cnt (x) b_fc
                agg = aggps.tile([P, D], f32, space="PSUM", tag="agg",
                                 name="agg")
                nc.tensor.matmul(out=agg[:], lhsT=sxT[:], rhs=w3[:, 0:D],
                                 start=True, stop=False)
                nc.tensor.matmul(out=agg[:], lhsT=diag[:], rhs=gl[:],
                                 start=False, stop=False)
                nc.tensor.matmul(out=agg[:], lhsT=cnt2[0:1, t * P:(t + 1) * P],
                                 rhs=bfcrow[0:1, :], start=False, stop=True)
                # rate = softplus(xw3[:,128:256]); 1/den via exp(-ln(den));
                # exp/ln share one ACT table so no table reloads
                esp = sbp.tile([P, D], f32, tag="esp", name="esp")
                nc.scalar.activation(out=esp[:], in_=xw3[:, D:2 * D],
                                     func=AF.Exp)
                rate = sbp.tile([P, D], f32, tag="rate", name="rate")
                nc.scalar.activation(out=rate[:], in_=esp[:], func=AF.Ln,
                                     bias=1.0)
                # 1/den = exp(-ln(den)), den = (rate+EPS)*deg + 1 + EPS
                invd = sbp.tile([P, D], f32, tag="invd", name="invd")
                if USE_ACT_APSCALE:
                    # ln(den) = ln(rate*deg + (eps*deg + 1 + eps)) in ONE op
                    nc.scalar.activation(out=invd[:], in_=rate[:], func=AF.Ln,
                                         scale=degf[:, t:t + 1],
                                         bias=degb2[:, t:t + 1])
                else:
                    degb = degf[:, t:t + 1].to_broadcast([P, D])
                    nc.vector.scalar_tensor_tensor(
                        out=invd[:], in0=rate[:], scalar=EPS, in1=degb,
                        op0=AO.add, op1=AO.mult)
                    nc.scalar.activation(out=invd[:], in_=invd[:], func=AF.Ln,
                                         bias=1.0 + EPS)
                nc.scalar.activation(out=invd[:], in_=invd[:], func=AF.Exp,
                                     scale=-1.0)
                # num = (rate+EPS)*agg + gamma
                num = sbp.tile([P, D], f32, tag="num", name="num")
                nc.vector.scalar_tensor_tensor(
                    out=num[:], in0=rate[:], scalar=EPS, in1=agg[:],
                    op0=AO.add, op1=AO.mult)
                nc.vector.tensor_add(out=num[:], in0=num[:],
                                     in1=xw3[:, 2 * D:3 * D])
                # y0 = num/den with row sums for the LN stats
                y0 = y0sg[:, tl, :]
                if USE_TTR:
                    nc.vector.tensor_tensor_reduce(
                        out=y0, in0=num[:], in1=invd[:], scale=1.0, scalar=0.0,
                        op0=AO.mult, op1=AO.add,
                        accum_out=s1[:, tl:tl + 1])
                    sq = sbp.tile([P, D], f32, tag="sq", name="sq")
                    nc.vector.tensor_tensor_reduce(
                        out=sq[:], in0=y0, in1=y0, scale=1.0, scalar=0.0,
                        op0=AO.mult, op1=AO.add,
                        accum_out=s2[:, tl:tl + 1])
                else:
                    nc.vector.tensor_mul(out=y0, in0=num[:], in1=invd[:])
                    sq = sbp.tile([P, D], f32, tag="sq", name="sq")
                    nc.scalar.square(sq[:], y0)
                    nc.vector.tensor_reduce(
                        out=s1[:, tl:tl + 1], in_=y0,
                        axis=mybir.AxisListType.X, op=AO.add)
                    nc.vector.tensor_reduce(
                        out=s2[:, tl:tl + 1], in_=sq[:],
                        axis=mybir.AxisListType.X, op=AO.add)

            # LayerNorm stats for the super-group's SG tiles at once
            nc.vector.tensor_scalar_mul(out=mean, in0=s1, scalar1=1.0 / D)
            nc.vector.tensor_scalar_mul(out=msq, in0=s2, scalar1=1.0 / D)
            nc.vector.tensor_tensor(out=var, in0=mean, in1=mean, op=AO.mult)
            nc.vector.tensor_sub(out=var, in0=msq, in1=var)
            nc.scalar.activation(out=var, in_=var, func=AF.Ln, bias=LN_EPS)
            nc.scalar.activation(out=rstd, in_=var, func=AF.Exp, scale=-0.5)
            nc.vector.tensor_tensor(out=mrs, in0=mean, in1=rstd, op=AO.mult)
            for tl in range(SG):
                yf = yt[:, tl, :]
                if USE_STT_AP:
                    nc.vector.scalar_tensor_tensor(
                        out=yf, in0=y0sg[:, tl, :], scalar=rstd[:, tl:tl + 1],
                        in1=mrs[:, tl:tl + 1].to_broadcast([P, D]),
                        op0=AO.mult, op1=AO.subtract)
                else:
                    meanb = mean[:, tl:tl + 1].to_broadcast([P, D])
                    rstdb = rstd[:, tl:tl + 1].to_broadcast([P, D])
                    nc.vector.tensor_sub(out=yf, in0=y0sg[:, tl, :], in1=meanb)
                    nc.vector.tensor_mul(out=yf, in0=yf, in1=rstdb)
                if lnab is not None:
                    lg = lnab[:, 0:D]
                    lb = lnab[:, D:2 * D]
                    nc.vector.tensor_mul(out=yf, in0=yf, in1=lg)
                    nc.vector.tensor_add(out=yf, in0=yf, in1=lb)
            dst = d_y[t0 * P:(t0 + SG) * P, :].rearrange("(t p) d -> p t d",
                                                         p=P)
            nc.sync.dma_start(dst, yt[:])

    nc.compile()
    return nc


def run(inputs, cfg: Cfg, core_ids=None):
    in_maps = prep(**inputs, cfg=cfg)
    nc = build(cfg)
    res = run_bass_kernel_spmd(nc, in_maps, core_ids=core_ids or list(range(cfg.NC)))
    ys = [res.results[r]["y"][:cfg.NLOC] for r in range(cfg.NC)]
    return np.concatenate(ys, axis=0).astype(np.float32)


def kernel(**inputs):
    cfg = Cfg(N=100_000, E=800_000, NC=8)
    return run(inputs, cfg)


# revision 11
# speedup vs baseline: 2.2580x; 1.0230x over previous
"""Trainium2 Bass kernel for a GNN message-passing layer (BoundaryConvLayer).

Computation (reference, per node i over D=128 channels):
    rate  = softplus(x @ W_rate) + EPS
    gamma = x @ W_rob + b_rob
    h     = x @ W_fc + b_fc
    agg   = segment_sum(h[row] + h[col], row)
    y     = LayerNorm((rate*agg + gamma) / (1 + rate*deg + EPS)) * ln_gamma + ln_beta

Distribution: nodes sharded across 8 cores by contiguous row blocks; edges
partitioned by destination row so the segment sum is local to each core.

Key identity (g = x @ W_fc, cnt = in-edge count):
    agg[i] = (sum_{e:row=i} x[col_e]  +  cnt[i]*x[i]) @ W_fc + 2*cnt[i]*b_fc
The segment sum runs in INPUT space: the host stages the per-edge source
rows x[col_e] (pure indexing, no host FLOPs) grouped by destination tile,
and the PE reduces each 128-slot group with a one-hot "selection matrix"
matmul accumulated in PSUM; the self term is an extra slot group holding
the tile's own rows with sel = diag(cnt).  This removes the device-side
table gather (a software-DGE bottleneck) and the redundant full-N GEMM:
all DMA is large contiguous hardware-queue traffic.

Layout/throughput notes:
  - PSUM banks are packed (4 agg tiles or 2 xw2 tiles per 2KB bank) so the
    PSUM->SBUF copies are few and wide.
  - Elementwise + LayerNorm runs once per 7-tile super-group on [128, 896]
    operands, split across DVE / ACT / Pool.
  - 1/den and rsqrt go through exp/ln so one ACT table load suffices.
"""

import numpy as np
import ml_dtypes
from contextlib import ExitStack
from dataclasses import dataclass

import concourse.bass as bass
import concourse.tile as tile
from concourse import bacc, mybir
from concourse.bass_utils import run_bass_kernel_spmd

# The stock ACT-table chooser greedily picks the first set containing each
# function, which for {Exp, Ln, Copy} can alternate between two sets and
# reload the table per use (~1.3us each).  Restrict it to the one set that
# contains all of them so a single load suffices.
_ACT_KEEP = "natural_log_exp_and_others"
if not getattr(bacc, "_act_tables_patched", False):
    _orig_get_tables = bacc.get_activation_tables

    def _patched_get_tables(arch):
        t = _orig_get_tables(arch)
        if _ACT_KEEP in t:
            t = {k: (v if k == _ACT_KEEP else set()) for k, v in t.items()}
        return t

    bacc.get_activation_tables = _patched_get_tables
    bacc._act_tables_patched = True

BF16 = ml_dtypes.bfloat16
EPS = 1e-4
LN_EPS = 1e-5
P = 128
D = 128


@dataclass
class Cfg:
    N: int            # total nodes
    E: int            # total edges
    NC: int           # cores
    S: int = 0        # edge slot groups per tile (set by prep)
    SG: int = 7       # tiles per super-group (pipelining granule)
    ln_trivial: bool = False

    @property
    def NLOC(self):
        return self.N // self.NC

    @property
    def T(self):
        return (self.NLOC + P - 1) // P

    @property
    def TLP(self):
        return self.T * P

    @property
    def NSG(self):
        assert self.T % self.SG == 0
        return self.T // self.SG

    @property
    def SP1(self):    # slot groups incl. the self group
        return self.S + 1


def prep(x, edge_index, degree, W_fc, b_fc, W_rate, W_rob, b_rob, ln_gamma, ln_beta,
         cfg: Cfg):
    """Host-side preprocessing: shard + stage per-edge source rows by dst tile."""
    N, NC = cfg.N, cfg.NC
    NLOC, T, TLP = cfg.NLOC, cfg.T, cfg.TLP

    x = np.asarray(x, np.float32)
    edge_index = np.asarray(edge_index, np.int64)
    degree = np.asarray(degree)
    row, col = edge_index[0], edge_index[1]

    xbf = x.astype(BF16)
    xbf_ext = np.concatenate([xbf, np.zeros((1, D), BF16)], axis=0)  # pad row

    wfc = np.ascontiguousarray(W_fc, dtype=np.float32).astype(BF16)
    w2 = np.zeros((P, 2 * D), BF16)
    w2[:, 0:D] = np.asarray(W_rate, np.float32).astype(BF16)
    w2[:, D:2 * D] = np.asarray(W_rob, np.float32).astype(BF16)
    brow2 = np.zeros((1, 2 * D), np.float32)
    brow2[0, D:2 * D] = np.asarray(b_rob, np.float32)
    bfcrow = np.asarray(b_fc, np.float32).reshape(1, D).copy()
    onesrow = np.ones((1, D), np.float32)
    ident = np.eye(P, dtype=BF16)

    cfg.ln_trivial = bool(np.all(np.asarray(ln_gamma) == 1.0)
                          and np.all(np.asarray(ln_beta) == 0.0))
    lnab = np.zeros((P, 2 * D), np.float32)
    lnab[:, :D] = np.asarray(ln_gamma, np.float32)[None, :]
    lnab[:, D:] = np.asarray(ln_beta, np.float32)[None, :]

    core_of = row // NLOC

    # pass 1: per-core per-tile edge counts fix the global S (slot groups/tile)
    percore = []
    S = 1
    for r in range(NC):
        m = core_of == r
        rl = row[m] - r * NLOC
        ce = col[m]
        cnt = np.bincount(rl, minlength=TLP)
        cnt_t = np.bincount(rl // P, minlength=T)
        S = max(S, -(-int(cnt_t.max()) // P))
        percore.append((rl, ce, cnt, cnt_t))
    cfg.S = S
    SP1 = S + 1
    SPT = SP1 * P  # slots per tile incl. self group

    in_maps = []
    for r in range(NC):
        rl, ce, cnt, cnt_t = percore[r]
        # order edges by (tile, col) -> fill each tile's slots densely; the
        # col ordering gives the staging gather some source locality
        order = np.lexsort((ce, rl // P))
        rl_s, ce_s = rl[order], ce[order]
        t_s = rl_s // P
        run_start = np.zeros(T + 1, np.int64)
        np.cumsum(cnt_t, out=run_start[1:])
        pos = np.arange(len(rl_s)) - run_start[t_s]     # slot within tile
        slot = t_s * SPT + pos                           # global slot id

        # staged source rows, SBUF layout [128, T*SP1*128] bf16:
        # partition = slot % 128, free = (slot//128)*128 + d.
        # group S of each tile holds the tile's own 128 rows (self term).
        src = np.full(T * SPT, N, np.int64)              # pads -> zero row
        src[slot] = ce_s
        own = r * NLOC + np.arange(TLP)
        own[NLOC:] = N
        src.reshape(T, SP1, P)[:, S, :] = own.reshape(T, P)
        xe_sb = np.ascontiguousarray(
            xbf_ext[src].reshape(T * SP1, P, D).transpose(1, 0, 2)
        ).reshape(P, T * SP1 * D)

        # rowsr: dst-in-tile per edge slot, -1 for pads -> zero sel column
        rowsr = np.full((P, T * S), -1.0, BF16)
        rowsr[pos % P, t_s * S + pos // P] = (rl_s % P).astype(BF16)

        iotab = np.broadcast_to(
            np.arange(P, dtype=BF16)[None, None, :], (P, S, P)
        ).reshape(P, S * P).copy()

        cnt2 = (2.0 * cnt).astype(np.float32)[None, :]
        cntb = cnt.astype(np.float32).reshape(T, P).T.copy()
        degl = np.zeros(TLP, np.float32)
        degl[:NLOC] = degree[r * NLOC:(r + 1) * NLOC].astype(np.float32)
        degf = degl.reshape(T, P).T.copy()

        xTloc = np.zeros((P, TLP), BF16)
        xTloc[:, :NLOC] = xbf[r * NLOC:(r + 1) * NLOC].T

        in_maps.append({
            "xe": xe_sb, "xT": xTloc,
            "wfc": wfc, "w2": w2, "brow2": brow2, "bfcrow": bfcrow,
            "onesrow": onesrow, "ident": ident, "lnab": lnab,
            "iotab": iotab, "rowsr": rowsr,
            "cnt2": cnt2, "cntb": cntb, "degf": degf,
        })
    return in_maps


def build(cfg: Cfg):
    """Build the SPMD Bass program (identical on every core)."""
    NC, T, TLP = cfg.NC, cfg.T, cfg.TLP
    S, SP1, SG, NSG = cfg.S, cfg.SP1, cfg.SG, cfg.NSG
    SPT = SP1 * P
    bf = mybir.dt.bfloat16
    f32 = mybir.dt.float32
    AO = mybir.AluOpType
    AF = mybir.ActivationFunctionType

    nc = bacc.Bacc("TRN2", target_bir_lowering=False, debug=False, num_devices=NC)
    for val in (LN_EPS, 1.0 + EPS):
        cs = nc.alloc_sbuf_tensor(f"const-float32-{val}", [P, 1], f32)
        nc.gpsimd.memset(cs.ap(), val)
        nc.const_aps.aps[(f32, val)] = cs.ap()
    nc.all_engine_barrier()

    d_xe = nc.dram_tensor("xe", [P, T * SPT], bf, kind="ExternalInput").ap()
    d_xT = nc.dram_tensor("xT", [P, TLP], bf, kind="ExternalInput").ap()
    d_wfc = nc.dram_tensor("wfc", [P, D], bf, kind="ExternalInput").ap()
    d_w2 = nc.dram_tensor("w2", [P, 2 * D], bf, kind="ExternalInput").ap()
    d_brow2 = nc.dram_tensor("brow2", [1, 2 * D], f32, kind="ExternalInput").ap()
    d_bfc = nc.dram_tensor("bfcrow", [1, D], f32, kind="ExternalInput").ap()
    d_ones = nc.dram_tensor("onesrow", [1, D], f32, kind="ExternalInput").ap()
    d_ident = nc.dram_tensor("ident", [P, P], bf, kind="ExternalInput").ap()
    d_lnab = nc.dram_tensor("lnab", [P, 2 * D], f32, kind="ExternalInput").ap()
    d_iota = nc.dram_tensor("iotab", [P, S * P], bf, kind="ExternalInput").ap()
    d_rowsr = nc.dram_tensor("rowsr", [P, T * S], bf, kind="ExternalInput").ap()
    d_cnt2 = nc.dram_tensor("cnt2", [1, TLP], f32, kind="ExternalInput").ap()
    d_cntb = nc.dram_tensor("cntb", [P, T], f32, kind="ExternalInput").ap()
    d_degf = nc.dram_tensor("degf", [P, T], f32, kind="ExternalInput").ap()
    d_y = nc.dram_tensor("y", [TLP, D], bf, kind="ExternalOutput").ap()

    with tile.TileContext(nc) as tc, ExitStack() as ctx:
        from concourse import library_config
        nc.gpsimd.load_library(library_config.standard)
        consts = ctx.enter_context(tc.tile_pool(name="consts", bufs=1))
        wfc = consts.tile([P, D], bf)
        nc.sync.dma_start(wfc[:], d_wfc[:])
        w2 = consts.tile([P, 2 * D], bf)
        nc.sync.dma_start(w2[:], d_w2[:])
        xlocT = consts.tile([P, TLP], bf)
        nc.sync.dma_start(xlocT[:], d_xT[:])
        brow2 = consts.tile([1, 2 * D], f32)
        nc.sync.dma_start(brow2[:], d_brow2[:])
        bfcrow = consts.tile([1, D], f32)
        nc.sync.dma_start(bfcrow[:], d_bfc[:])
        onesr = consts.tile([1, D], f32)
        nc.sync.dma_start(onesr[:], d_ones[:])
        ident = consts.tile([P, P], bf)
        nc.sync.dma_start(ident[:], d_ident[:])
        iota = consts.tile([P, S * P], bf)
        nc.sync.dma_start(iota[:], d_iota[:])
        rowsr = consts.tile([P, T * S], bf)
        nc.sync.dma_start(rowsr[:], d_rowsr[:])
        cnt2 = consts.tile([1, TLP], f32)
        nc.sync.dma_start(cnt2[:], d_cnt2[:])
        cntb = consts.tile([P, T], f32)
        nc.sync.dma_start(cntb[:], d_cntb[:])
        degf = consts.tile([P, T], f32)
        nc.sync.dma_start(degf[:], d_degf[:])
        lnab = None
        if not cfg.ln_trivial:
            lnab = consts.tile([P, 2 * D], f32)
            nc.sync.dma_start(lnab[:], d_lnab[:])

        xep = ctx.enter_context(tc.tile_pool(name="xep", bufs=3))
        selp = ctx.enter_context(tc.tile_pool(name="selp", bufs=4))
        xw2ps = ctx.enter_context(tc.tile_pool(name="xw2ps", bufs=3, space="PSUM"))
        sxps = ctx.enter_context(tc.tile_pool(name="sxps", bufs=2, space="PSUM"))
        aggps = ctx.enter_context(tc.tile_pool(name="aggps", bufs=2, space="PSUM"))
        sxtp = ctx.enter_context(tc.tile_pool(name="sxtp", bufs=3))
        sgp = ctx.enter_context(tc.tile_pool(name="sgp", bufs=2))
        ysgp = ctx.enter_context(tc.tile_pool(name="ysgp", bufs=2))
        stp = ctx.enter_context(tc.tile_pool(name="stp", bufs=2))

        NPAIR = (SG + 1) // 2   # xw2 pairs per SG (2 tiles / PSUM bank)
        NQUAD = (SG + 3) // 4   # sx/agg quads per SG (4 tiles / PSUM bank)

        for sg in range(NSG):
            t0 = sg * SG
            xe = xep.tile([P, SG * SPT], bf, tag="xe", name="xe")
            nc.sync.dma_start(xe[:], d_xe[:, t0 * SPT:(t0 + SG) * SPT])

            rate_sg = sgp.tile([P, SG, D], f32, tag="rate", name="rate")
            gam_sg = sgp.tile([P, SG, D], f32, tag="gam", name="gam")
            agg_sg = sgp.tile([P, SG, D], f32, tag="agg", name="agg")
            num_sg = sgp.tile([P, SG, D], f32, tag="num", name="num")
            invd_sg = sgp.tile([P, SG, D], f32, tag="invd", name="invd")
            y0_sg = sgp.tile([P, SG, D], f32, tag="y0", name="y0")
            yt = ysgp.tile([P, SG, D], bf, tag="yt", name="yt")
            st = stp.tile([P, 8 * SG], f32, tag="st", name="st")
            s1 = st[:, 0 * SG:1 * SG]
            s2 = st[:, 1 * SG:2 * SG]
            mean = st[:, 2 * SG:3 * SG]
            msq = st[:, 3 * SG:4 * SG]
            var = st[:, 4 * SG:5 * SG]
            rstd = st[:, 5 * SG:6 * SG]

            # ---- pass 1: weights GEMMs, sel build, x-space segment sums ----
            sels = []
            sxs = []
            xw2b = None
            for tl in range(SG):
                t = t0 + tl
                pi = tl % 2
                if pi == 0:
                    pw = min(2, SG - tl)
                    xw2b = xw2ps.tile([P, 2, 2 * D], f32, space="PSUM",
                                      tag="xw2", name="xw2")
                nc.tensor.matmul(out=xw2b[:, pi, :],
                                 lhsT=xlocT[:, t * P:(t + 1) * P],
                                 rhs=w2[:], start=True, stop=False)
                nc.tensor.matmul(out=xw2b[:, pi, :], lhsT=onesr[0:1, :],
                                 rhs=brow2[0:1, :], start=False, stop=True)
                sel = selp.tile([P, SPT], bf, tag="sel", name="sel")
                sels.append(sel)
                rb = rowsr[:, t * S:(t + 1) * S][:, :, None] \
                    .to_broadcast([P, S, P])
                nc.vector.tensor_tensor(
                    out=sel[:, 0:S * P].rearrange("p (s m) -> p s m", s=S),
                    in0=rb, in1=iota.rearrange("p (s m) -> p s m", s=S),
                    op=AO.is_equal)
                nc.gpsimd.tensor_tensor(
                    out=sel[:, S * P:SP1 * P], in0=ident[:],
                    in1=cntb[:, t:t + 1].to_broadcast([P, P]), op=AO.mult)
                qi = tl % 4
                if qi == 0:
                    qw = min(4, SG - tl)
                    sxb = sxps.tile([P, 4, D], f32, space="PSUM", tag="sx",
                                    name="sx")
                    sxs.append((sxb, qw))
                for s in range(SP1):
                    g0 = (tl * SP1 + s) * D
                    nc.tensor.matmul(out=sxb[:, qi, :], lhsT=xe[:, g0:g0 + D],
                                     rhs=sel[:, s * P:(s + 1) * P],
                                     start=(s == 0), stop=(s == SP1 - 1))
                if pi + 1 == pw:
                    # rate/gamma for the completed pair
                    b0 = tl - pi
                    pre = xw2b[:, 0:pw, 0:D]
                    gam = xw2b[:, 0:pw, D:2 * D]
                    nc.scalar.activation(out=rate_sg[:, b0:b0 + pw, :],
                                         in_=pre, func=AF.Exp)
                    nc.vector.tensor_copy(out=gam_sg[:, b0:b0 + pw, :],
                                          in_=gam)

            nc.scalar.activation(out=rate_sg[:], in_=rate_sg[:], func=AF.Ln,
                                 bias=1.0)

            # ---- pass 2: agg GEMMs per quad ----
            for q in range(NQUAD):
                sxb, qw = sxs[q]
                sxT = sxtp.tile([P, 4, D], bf, tag="sxT", name="sxT")
                nc.scalar.copy(sxT[:, 0:qw, :], sxb[:, 0:qw, :])
                aggb = aggps.tile([P, 4, D], f32, space="PSUM", tag="aggb",
                                  name="aggb")
                for i in range(qw):
                    t = t0 + q * 4 + i
                    nc.tensor.matmul(out=aggb[:, i, :], lhsT=sxT[:, i, :],
                                     rhs=wfc[:], start=True, stop=False)
                    nc.tensor.matmul(out=aggb[:, i, :],
                                     lhsT=cnt2[0:1, t * P:(t + 1) * P],
                                     rhs=bfcrow[0:1, :], start=False, stop=True)
                nc.scalar.copy(agg_sg[:, q * 4:q * 4 + qw, :], aggb[:, 0:qw, :])

            # ---- pass 3: batched elementwise + LayerNorm ----
            degb = degf[:, t0:t0 + SG][:, :, None].to_broadcast([P, SG, D])
            nc.vector.scalar_tensor_tensor(
                out=invd_sg[:], in0=rate_sg[:], scalar=EPS, in1=degb,
                op0=AO.add, op1=AO.mult)
            nc.scalar.activation(out=invd_sg[:], in_=invd_sg[:], func=AF.Ln,
                                 bias=1.0 + EPS)
            nc.scalar.activation(out=invd_sg[:], in_=invd_sg[:], func=AF.Exp,
                                 scale=-1.0)
            nc.vector.scalar_tensor_tensor(
                out=num_sg[:], in0=rate_sg[:], scalar=EPS, in1=agg_sg[:],
                op0=AO.add, op1=AO.mult)
            nc.vector.tensor_add(out=num_sg[:], in0=num_sg[:], in1=gam_sg[:])
            nc.gpsimd.tensor_tensor(out=y0_sg[:], in0=num_sg[:],
                                    in1=invd_sg[:], op=AO.mult)
            nc.vector.tensor_reduce(out=s1, in_=y0_sg[:],
                                    axis=mybir.AxisListType.X, op=AO.add)
            # reuse num_sg as the square scratch
            nc.gpsimd.tensor_tensor(out=num_sg[:], in0=y0_sg[:], in1=y0_sg[:],
                                    op=AO.mult)
            nc.vector.tensor_reduce(out=s2, in_=num_sg[:],
                                    axis=mybir.AxisListType.X, op=AO.add)
            nc.vector.tensor_scalar_mul(out=mean, in0=s1, scalar1=1.0 / D)
            nc.vector.tensor_scalar_mul(out=msq, in0=s2, scalar1=1.0 / D)
            nc.vector.tensor_tensor(out=var, in0=mean, in1=mean, op=AO.mult)
            nc.vector.tensor_sub(out=var, in0=msq, in1=var)
            nc.scalar.activation(out=var, in_=var, func=AF.Ln, bias=LN_EPS)
            nc.scalar.activation(out=rstd, in_=var, func=AF.Exp, scale=-0.5)
            meanb = mean[:, :, None].to_broadcast([P, SG, D])
            rstdb = rstd[:, :, None].to_broadcast([P, SG, D])
            nc.vector.tensor_sub(out=y0_sg[:], in0=y0_sg[:], in1=meanb)
            if lnab is None:
                nc.gpsimd.tensor_tensor(out=yt[:], in0=y0_sg[:], in1=rstdb,
                                        op=AO.mult)
            else:
                nc.gpsimd.tensor_tensor(out=y0_sg[:], in0=y0_sg[:], in1=rstdb,
                                        op=AO.mult)
                lg = lnab[:, 0:D][:, None, :].to_broadcast([P, SG, D])
                lb = lnab[:, D:2 * D][:, None, :].to_broadcast([P, SG, D])
                nc.vector.tensor_mul(out=y0_sg[:], in0=y0_sg[:], in1=lg)
                nc.vector.tensor_add(out=yt[:], in0=y0_sg[:], in1=lb)
            dst = d_y[t0 * P:(t0 + SG) * P, :].rearrange("(t p) d -> p t d",
                                                         p=P)
            nc.sync.dma_start(dst, yt[:])

    nc.compile()
    return nc


def run(inputs, cfg: Cfg, core_ids=None):
    in_maps = prep(**inputs, cfg=cfg)
    nc = build(cfg)
    res = run_bass_kernel_spmd(nc, in_maps, core_ids=core_ids or list(range(cfg.NC)))
    ys = [res.results[r]["y"][:cfg.NLOC] for r in range(cfg.NC)]
    return np.concatenate(ys, axis=0).astype(np.float32)


def kernel(**inputs):
    cfg = Cfg(N=100_000, E=800_000, NC=8)
    return run(inputs, cfg)


# revision 21
# speedup vs baseline: 2.4931x; 1.1041x over previous
"""Trainium2 Bass kernel for a GNN message-passing layer (BoundaryConvLayer).

Computation (reference, per node i over D=128 channels):
    rate  = softplus(x @ W_rate) + EPS
    gamma = x @ W_rob + b_rob
    h     = x @ W_fc + b_fc
    agg   = segment_sum(h[row] + h[col], row)
    y     = LayerNorm((rate*agg + gamma) / (1 + rate*deg + EPS)) * ln_gamma + ln_beta

Distribution: nodes sharded across 8 cores by contiguous row blocks; edges
partitioned by destination row so the segment sum is local to each core.

Key identity (g = x @ W_fc, cnt = in-edge count):
    agg[i] = (sum_{e:row=i} x[col_e]  +  cnt[i]*x[i]) @ W_fc + 2*cnt[i]*b_fc
The segment sum runs in INPUT space: the host stages the per-edge source
rows x[col_e] (pure indexing, no host FLOPs) grouped by destination tile,
and the PE reduces each 128-slot group with a one-hot "selection matrix"
matmul accumulated in PSUM; the self term is an extra slot group holding
the tile's own rows with sel = diag(cnt).  This removes the device-side
table gather (a software-DGE bottleneck) and the redundant full-N GEMM:
all DMA is large contiguous hardware-queue traffic.

Throughput notes:
  - Software pipelined: super-group g's elementwise/LayerNorm is emitted
    AFTER super-group g+1's matmul work, so the PE never starves at SG
    boundaries waiting for DVE to finish the previous tail.
  - PSUM banks are packed (4 agg tiles / 2 xw2 tiles per 2KB bank) so the
    PSUM->SBUF copies are few and wide; the agg phase of a completed quad
    is interleaved one tile later to hide the copy latency.
  - Elementwise runs on [128, 896] bf16 operands (DVE 2x mode where the
    access pattern allows), reductions on Pool, exp/ln chains on ACT.
  - 1/den and rsqrt go through exp/ln so one ACT table load suffices.
"""

import numpy as np
import ml_dtypes
from contextlib import ExitStack
from dataclasses import dataclass

import concourse.bass as bass
import concourse.tile as tile
from concourse import bacc, mybir
from concourse.bass_utils import run_bass_kernel_spmd

# The stock ACT-table chooser greedily picks the first set containing each
# function, which for {Exp, Ln, Copy, Square} can alternate between two sets
# and reload the table per use (~1.3us each).  Restrict it to the one set
# that contains all of them so a single load suffices.
_ACT_KEEP = "natural_log_exp_and_others"
if not getattr(bacc, "_act_tables_patched", False):
    _orig_get_tables = bacc.get_activation_tables

    def _patched_get_tables(arch):
        t = _orig_get_tables(arch)
        if _ACT_KEEP in t:
            t = {k: (v if k == _ACT_KEEP else set()) for k, v in t.items()}
        return t

    bacc.get_activation_tables = _patched_get_tables
    bacc._act_tables_patched = True

BF16 = ml_dtypes.bfloat16
EPS = 1e-4
LN_EPS = 1e-5
P = 128
D = 128


@dataclass
class Cfg:
    N: int            # total nodes
    E: int            # total edges
    NC: int           # cores
    S: int = 0        # edge slot groups per tile (set by prep)
    SG: int = 7       # tiles per super-group (pipelining granule)
    ln_trivial: bool = False

    @property
    def NLOC(self):
        return self.N // self.NC

    @property
    def T(self):
        return (self.NLOC + P - 1) // P

    @property
    def TLP(self):
        return self.T * P

    @property
    def NSG(self):
        assert self.T % self.SG == 0
        return self.T // self.SG

    @property
    def SP1(self):    # slot groups incl. the self group
        return self.S + 1


def prep(x, edge_index, degree, W_fc, b_fc, W_rate, W_rob, b_rob, ln_gamma, ln_beta,
         cfg: Cfg):
    """Host-side preprocessing: shard + stage per-edge source rows by dst tile."""
    N, NC = cfg.N, cfg.NC
    NLOC, T, TLP = cfg.NLOC, cfg.T, cfg.TLP

    x = np.asarray(x, np.float32)
    edge_index = np.asarray(edge_index, np.int64)
    degree = np.asarray(degree)
    row, col = edge_index[0], edge_index[1]

    xbf = x.astype(BF16)
    xbf_ext = np.concatenate([xbf, np.zeros((1, D), BF16)], axis=0)  # pad row

    wfc = np.ascontiguousarray(W_fc, dtype=np.float32).astype(BF16)
    w2 = np.zeros((P, 2 * D), BF16)
    w2[:, 0:D] = np.asarray(W_rate, np.float32).astype(BF16)
    w2[:, D:2 * D] = np.asarray(W_rob, np.float32).astype(BF16)
    brob = np.zeros((1, 2 * D), np.float32)
    brob[0, D:2 * D] = np.asarray(b_rob, np.float32)
    bfcrow = np.asarray(b_fc, np.float32).reshape(1, D).copy()
    onesrow = np.ones((1, D), np.float32)
    ident = np.eye(P, dtype=BF16)

    cfg.ln_trivial = bool(np.all(np.asarray(ln_gamma) == 1.0)
                          and np.all(np.asarray(ln_beta) == 0.0))
    lnab = np.zeros((P, 2 * D), np.float32)
    lnab[:, :D] = np.asarray(ln_gamma, np.float32)[None, :]
    lnab[:, D:] = np.asarray(ln_beta, np.float32)[None, :]

    core_of = row // NLOC

    # pass 1: per-core per-tile edge counts fix the global S (slot groups/tile)
    percore = []
    S = 1
    for r in range(NC):
        m = core_of == r
        rl = row[m] - r * NLOC
        ce = col[m]
        cnt = np.bincount(rl, minlength=TLP)
        cnt_t = np.bincount(rl // P, minlength=T)
        S = max(S, -(-int(cnt_t.max()) // P))
        percore.append((rl, ce, cnt, cnt_t))
    cfg.S = S
    SP1 = S + 1
    SPT = SP1 * P  # slots per tile incl. self group

    in_maps = []
    for r in range(NC):
        rl, ce, cnt, cnt_t = percore[r]
        # order edges by (tile, col) -> fill each tile's slots densely; the
        # col ordering gives the staging gather some source locality
        order = np.lexsort((ce, rl // P))
        rl_s, ce_s = rl[order], ce[order]
        t_s = rl_s // P
        run_start = np.zeros(T + 1, np.int64)
        np.cumsum(cnt_t, out=run_start[1:])
        pos = np.arange(len(rl_s)) - run_start[t_s]     # slot within tile
        slot = t_s * SPT + pos                           # global slot id

        # staged source rows, SBUF layout [128, T*SP1*128] bf16:
        # partition = slot % 128, free = (slot//128)*128 + d.
        # group S of each tile holds the tile's own 128 rows (self term).
        src = np.full(T * SPT, N, np.int64)              # pads -> zero row
        src[slot] = ce_s
        own = r * NLOC + np.arange(TLP)
        own[NLOC:] = N
        src.reshape(T, SP1, P)[:, S, :] = own.reshape(T, P)
        xe_sb = np.ascontiguousarray(
            xbf_ext[src].reshape(T * SP1, P, D).transpose(1, 0, 2)
        ).reshape(P, T * SP1 * D)

        # rowsr: dst-in-tile per edge slot, -1 for pads -> zero sel column
        rowsr = np.full((P, T * S), -1.0, BF16)
        rowsr[pos % P, t_s * S + pos // P] = (rl_s % P).astype(BF16)

        iotab = np.broadcast_to(
            np.arange(P, dtype=BF16)[None, None, :], (P, S, P)
        ).reshape(P, S * P).copy()

        cnt2 = (2.0 * cnt).astype(np.float32)[None, :]
        cntb = cnt.astype(np.float32).reshape(T, P).T.copy()
        degl = np.zeros(TLP, np.float32)
        degl[:NLOC] = degree[r * NLOC:(r + 1) * NLOC].astype(np.float32)
        degf = degl.reshape(T, P).T.copy()

        xTloc = np.zeros((P, TLP), BF16)
        xTloc[:, :NLOC] = xbf[r * NLOC:(r + 1) * NLOC].T

        in_maps.append({
            "xe": xe_sb, "xT": xTloc,
            "wfc": wfc, "w2": w2, "brob": brob, "bfcrow": bfcrow,
            "onesrow": onesrow, "ident": ident, "lnab": lnab,
            "iotab": iotab, "rowsr": rowsr,
            "cnt2": cnt2, "cntb": cntb, "degf": degf,
        })
    return in_maps


def build(cfg: Cfg):
    """Build the SPMD Bass program (identical on every core)."""
    NC, T, TLP = cfg.NC, cfg.T, cfg.TLP
    S, SP1, SG, NSG = cfg.S, cfg.SP1, cfg.SG, cfg.NSG
    SPT = SP1 * P
    bf = mybir.dt.bfloat16
    f32 = mybir.dt.float32
    AO = mybir.AluOpType
    AF = mybir.ActivationFunctionType

    nc = bacc.Bacc("TRN2", target_bir_lowering=False, debug=False, num_devices=NC)
    for val in (LN_EPS, 1.0 + EPS):
        cs = nc.alloc_sbuf_tensor(f"const-float32-{val}", [P, 1], f32)
        nc.gpsimd.memset(cs.ap(), val)
        nc.const_aps.aps[(f32, val)] = cs.ap()
    nc.all_engine_barrier()

    d_xe = nc.dram_tensor("xe", [P, T * SPT], bf, kind="ExternalInput").ap()
    d_xT = nc.dram_tensor("xT", [P, TLP], bf, kind="ExternalInput").ap()
    d_wfc = nc.dram_tensor("wfc", [P, D], bf, kind="ExternalInput").ap()
    d_w2 = nc.dram_tensor("w2", [P, 2 * D], bf, kind="ExternalInput").ap()
    d_brob = nc.dram_tensor("brob", [1, 2 * D], f32, kind="ExternalInput").ap()
    d_bfc = nc.dram_tensor("bfcrow", [1, D], f32, kind="ExternalInput").ap()
    d_ones = nc.dram_tensor("onesrow", [1, D], f32, kind="ExternalInput").ap()
    d_ident = nc.dram_tensor("ident", [P, P], bf, kind="ExternalInput").ap()
    d_lnab = nc.dram_tensor("lnab", [P, 2 * D], f32, kind="ExternalInput").ap()
    d_iota = nc.dram_tensor("iotab", [P, S * P], bf, kind="ExternalInput").ap()
    d_rowsr = nc.dram_tensor("rowsr", [P, T * S], bf, kind="ExternalInput").ap()
    d_cnt2 = nc.dram_tensor("cnt2", [1, TLP], f32, kind="ExternalInput").ap()
    d_cntb = nc.dram_tensor("cntb", [P, T], f32, kind="ExternalInput").ap()
    d_degf = nc.dram_tensor("degf", [P, T], f32, kind="ExternalInput").ap()
    d_y = nc.dram_tensor("y", [TLP, D], bf, kind="ExternalOutput").ap()

    with tile.TileContext(nc) as tc, ExitStack() as ctx:
        from concourse import library_config
        nc.gpsimd.load_library(library_config.standard)
        consts = ctx.enter_context(tc.tile_pool(name="consts", bufs=1))
        wfc = consts.tile([P, D], bf)
        nc.sync.dma_start(wfc[:], d_wfc[:])
        w2 = consts.tile([P, 2 * D], bf)
        nc.sync.dma_start(w2[:], d_w2[:])
        xlocT = consts.tile([P, TLP], bf)
        nc.sync.dma_start(xlocT[:], d_xT[:])
        brob = consts.tile([1, 2 * D], f32)
        nc.sync.dma_start(brob[:], d_brob[:])
        bfcrow = consts.tile([1, D], f32)
        nc.sync.dma_start(bfcrow[:], d_bfc[:])
        onesr = consts.tile([1, D], f32)
        nc.sync.dma_start(onesr[:], d_ones[:])
        ident = consts.tile([P, P], bf)
        nc.sync.dma_start(ident[:], d_ident[:])
        iota = consts.tile([P, S * P], bf)
        nc.sync.dma_start(iota[:], d_iota[:])
        rowsr = consts.tile([P, T * S], bf)
        nc.sync.dma_start(rowsr[:], d_rowsr[:])
        cnt2 = consts.tile([1, TLP], f32)
        nc.sync.dma_start(cnt2[:], d_cnt2[:])
        cntb = consts.tile([P, T], f32)
        nc.sync.dma_start(cntb[:], d_cntb[:])
        degf = consts.tile([P, T], f32)
        nc.sync.dma_start(degf[:], d_degf[:])
        lnab = None
        if not cfg.ln_trivial:
            lnab = consts.tile([P, 2 * D], f32)
            nc.sync.dma_start(lnab[:], d_lnab[:])

        xep = ctx.enter_context(tc.tile_pool(name="xep", bufs=3))
        selp = ctx.enter_context(tc.tile_pool(name="selp", bufs=4))
        xw2ps = ctx.enter_context(tc.tile_pool(name="xw2ps", bufs=3, space="PSUM"))
        sxps = ctx.enter_context(tc.tile_pool(name="sxps", bufs=2, space="PSUM"))
        aggps = ctx.enter_context(tc.tile_pool(name="aggps", bufs=2, space="PSUM"))
        sxtp = ctx.enter_context(tc.tile_pool(name="sxtp", bufs=3))
        sgp = ctx.enter_context(tc.tile_pool(name="sgp", bufs=2))
        ysgp = ctx.enter_context(tc.tile_pool(name="ysgp", bufs=2))
        stp = ctx.enter_context(tc.tile_pool(name="stp", bufs=2))

        def emit_compute(sg):
            """Pass 1+2 for super-group sg: GEMMs, sel, x-space segment sums,
            agg.  Returns the context needed by the (deferred) elementwise."""
            t0 = sg * SG
            xe = xep.tile([P, SG * SPT], bf, tag="xe", name="xe")
            nc.sync.dma_start(xe[:], d_xe[:, t0 * SPT:(t0 + SG) * SPT])

            rate_sg = sgp.tile([P, SG, D], bf, tag="rate", name="rate")
            gam_sg = sgp.tile([P, SG, D], bf, tag="gam", name="gam")
            agg_sg = sgp.tile([P, SG, D], bf, tag="agg", name="agg")

            quads = []       # (sx psum bank, quad width, start tile)
            pending = []     # completed quads awaiting their agg phase

            def flush_quad():
                sxb, qw, q0 = pending.pop(0)
                sxT = sxtp.tile([P, 4, D], bf, tag="sxT", name="sxT")
                nc.scalar.copy(sxT[:, 0:qw, :], sxb[:, 0:qw, :])
                aggb = aggps.tile([P, 4, D], f32, space="PSUM", tag="aggb",
                                  name="aggb")
                for i in range(qw):
                    t = t0 + q0 + i
                    nc.tensor.matmul(out=aggb[:, i, :], lhsT=sxT[:, i, :],
                                     rhs=wfc[:], start=True, stop=False)
                    nc.tensor.matmul(out=aggb[:, i, :],
                                     lhsT=cnt2[0:1, t * P:(t + 1) * P],
                                     rhs=bfcrow[0:1, :], start=False, stop=True)
                nc.scalar.copy(agg_sg[:, q0:q0 + qw, :], aggb[:, 0:qw, :])

            xw2b = None
            sxb = None
            for tl in range(SG):
                t = t0 + tl
                pi = tl % 2
                if pi == 0:
                    pw = min(2, SG - tl)
                    xw2b = xw2ps.tile([P, 2, 2 * D], f32, space="PSUM",
                                      tag="xw2", name="xw2")
                nc.tensor.matmul(out=xw2b[:, pi, :],
                                 lhsT=xlocT[:, t * P:(t + 1) * P],
                                 rhs=w2[:], start=True, stop=False)
                nc.tensor.matmul(out=xw2b[:, pi, :], lhsT=onesr[0:1, :],
                                 rhs=brob[0:1, :], start=False, stop=True)
                sel = selp.tile([P, SPT], bf, tag="sel", name="sel")
                rb = rowsr[:, t * S:(t + 1) * S][:, :, None] \
                    .to_broadcast([P, S, P])
                nc.vector.tensor_tensor(
                    out=sel[:, 0:S * P].rearrange("p (s m) -> p s m", s=S),
                    in0=rb, in1=iota.rearrange("p (s m) -> p s m", s=S),
                    op=AO.is_equal)
                nc.vector.tensor_scalar_mul(
                    out=sel[:, S * P:SP1 * P], in0=ident[:],
                    scalar1=cntb[:, t:t + 1])
                qi = tl % 4
                if qi == 0:
                    qw = min(4, SG - tl)
                    sxb = sxps.tile([P, 4, D], f32, space="PSUM", tag="sx",
                                    name="sx")
                    quads.append((sxb, qw, tl))
                for s in range(SP1):
                    g0 = (tl * SP1 + s) * D
                    nc.tensor.matmul(out=sxb[:, qi, :], lhsT=xe[:, g0:g0 + D],
                                     rhs=sel[:, s * P:(s + 1) * P],
                                     start=(s == 0), stop=(s == SP1 - 1))
                if tl == quads[-1][2] + quads[-1][1] - 1:
                    pending.append(quads[-1])
                if pi + 1 == pw:
                    # rate/gamma for the completed pair
                    b0 = tl - pi
                    nc.scalar.activation(out=rate_sg[:, b0:b0 + pw, :],
                                         in_=xw2b[:, 0:pw, 0:D], func=AF.Exp)
                    nc.scalar.copy(gam_sg[:, b0:b0 + pw, :],
                                   xw2b[:, 0:pw, D:2 * D])
                # delay each quad's agg phase by one tile to hide the
                # PSUM->SBUF copy latency from the PE
                if pending and tl >= pending[0][2] + pending[0][1]:
                    flush_quad()
            while pending:
                flush_quad()
            nc.scalar.activation(out=rate_sg[:], in_=rate_sg[:], func=AF.Ln,
                                 bias=1.0)
            return dict(t0=t0, rate=rate_sg, gam=gam_sg, agg=agg_sg)

        def emit_eltwise(cx):
            """Pass 3 for a super-group: batched elementwise + LayerNorm."""
            t0 = cx["t0"]
            rate_sg, gam_sg, agg_sg = cx["rate"], cx["gam"], cx["agg"]
            num_sg = sgp.tile([P, SG, D], bf, tag="num", name="num")
            invd_sg = sgp.tile([P, SG, D], bf, tag="invd", name="invd")
            y0_sg = sgp.tile([P, SG, D], bf, tag="y0", name="y0")
            yt = ysgp.tile([P, SG, D], bf, tag="yt", name="yt")
            st = stp.tile([P, 6 * SG], f32, tag="st", name="st")
            stb = stp.tile([P, 2 * SG], bf, tag="stb", name="stb")
            s1 = st[:, 0 * SG:1 * SG]
            s2 = st[:, 1 * SG:2 * SG]
            mean = st[:, 2 * SG:3 * SG]
            rstd = st[:, 3 * SG:4 * SG]
            msq = st[:, 4 * SG:5 * SG]
            var = st[:, 5 * SG:6 * SG]
            meanb_src = stb[:, 0:SG]
            rstdb_src = stb[:, SG:2 * SG]

            degb = degf[:, t0:t0 + SG][:, :, None].to_broadcast([P, SG, D])
            nc.vector.scalar_tensor_tensor(
                out=invd_sg[:], in0=rate_sg[:], scalar=EPS, in1=degb,
                op0=AO.add, op1=AO.mult)
            nc.scalar.activation(out=invd_sg[:], in_=invd_sg[:], func=AF.Ln,
                                 bias=1.0 + EPS)
            nc.scalar.activation(out=invd_sg[:], in_=invd_sg[:], func=AF.Exp,
                                 scale=-1.0)
            nc.vector.scalar_tensor_tensor(
                out=num_sg[:], in0=rate_sg[:], scalar=EPS, in1=agg_sg[:],
                op0=AO.add, op1=AO.mult)
            nc.vector.tensor_add(out=num_sg[:], in0=num_sg[:], in1=gam_sg[:])
            nc.vector.tensor_mul(out=y0_sg[:], in0=num_sg[:], in1=invd_sg[:])
            # LayerNorm stats per (node, tile)
            nc.vector.tensor_reduce(out=s1, in_=y0_sg[:],
                                    axis=mybir.AxisListType.X, op=AO.add)
            # reuse num_sg as the square scratch
            nc.vector.tensor_mul(out=num_sg[:], in0=y0_sg[:], in1=y0_sg[:])
            nc.vector.tensor_reduce(out=s2, in_=num_sg[:],
                                    axis=mybir.AxisListType.X, op=AO.add)
            nc.vector.tensor_scalar_mul(out=mean, in0=s1, scalar1=1.0 / D)
            nc.vector.tensor_scalar_mul(out=msq, in0=s2, scalar1=1.0 / D)
            nc.vector.tensor_tensor(out=var, in0=mean, in1=mean, op=AO.mult)
            nc.vector.tensor_sub(out=var, in0=msq, in1=var)
            nc.scalar.activation(out=var, in_=var, func=AF.Ln, bias=LN_EPS)
            nc.scalar.activation(out=rstd, in_=var, func=AF.Exp, scale=-0.5)
            nc.scalar.copy(stb[:], st[:, 2 * SG:4 * SG])
            meanb = meanb_src[:, :, None].to_broadcast([P, SG, D])
            rstdb = rstdb_src[:, :, None].to_broadcast([P, SG, D])
            nc.vector.tensor_sub(out=y0_sg[:], in0=y0_sg[:], in1=meanb)
            if lnab is None:
                nc.vector.tensor_mul(out=yt[:], in0=y0_sg[:], in1=rstdb)
            else:
                nc.vector.tensor_mul(out=y0_sg[:], in0=y0_sg[:], in1=rstdb)
                lg = lnab[:, 0:D][:, None, :].to_broadcast([P, SG, D])
                lb = lnab[:, D:2 * D][:, None, :].to_broadcast([P, SG, D])
                nc.vector.tensor_mul(out=y0_sg[:], in0=y0_sg[:], in1=lg)
                nc.vector.tensor_add(out=yt[:], in0=y0_sg[:], in1=lb)
            dst = d_y[t0 * P:(t0 + SG) * P, :].rearrange("(t p) d -> p t d",
                                                         p=P)
            nc.sync.dma_start(dst, yt[:])

        prev = None
        for sg in range(NSG):
            cx = emit_compute(sg)
            if prev is not None:
                emit_eltwise(prev)
            prev = cx
        emit_eltwise(prev)

    nc.compile()
    return nc


def run(inputs, cfg: Cfg, core_ids=None):
    in_maps = prep(**inputs, cfg=cfg)
    nc = build(cfg)
    res = run_bass_kernel_spmd(nc, in_maps, core_ids=core_ids or list(range(cfg.NC)))
    ys = [res.results[r]["y"][:cfg.NLOC] for r in range(cfg.NC)]
    return np.concatenate(ys, axis=0).astype(np.float32)


def kernel(**inputs):
    cfg = Cfg(N=100_000, E=800_000, NC=8)
    return run(inputs, cfg)


# revision 25
# speedup vs baseline: 2.6363x; 1.0575x over previous
"""Trainium2 Bass kernel for a GNN message-passing layer (BoundaryConvLayer).

Computation (reference, per node i over D=128 channels):
    rate  = softplus(x @ W_rate) + EPS
    gamma = x @ W_rob + b_rob
    h     = x @ W_fc + b_fc
    agg   = segment_sum(h[row] + h[col], row)
    y     = LayerNorm((rate*agg + gamma) / (1 + rate*deg + EPS)) * ln_gamma + ln_beta

Distribution: nodes sharded across 8 cores by contiguous row blocks; edges
partitioned by destination row so the segment sum is local to each core.

Key identity (g = x @ W_fc, cnt = in-edge count):
    agg[i] = (sum_{e:row=i} x[col_e]  +  cnt[i]*x[i]) @ W_fc + 2*cnt[i]*b_fc
The segment sum runs in INPUT space: the host stages the per-edge source
rows x[col_e] (pure indexing, no host FLOPs) grouped by destination tile,
and the PE reduces each 128-slot group with a one-hot "selection matrix"
matmul accumulated in PSUM; the self term is an extra slot group holding
the tile's own rows with sel = diag(cnt).  This removes the device-side
table gather (a software-DGE bottleneck) and the redundant full-N GEMM:
all DMA is large contiguous hardware-queue traffic.

Throughput notes:
  - Software pipelined: super-group g's elementwise/LayerNorm is emitted
    AFTER super-group g+1's matmul work, so the PE never starves at SG
    boundaries waiting for DVE to finish the previous tail.
  - PSUM banks are packed (4 agg tiles / 2 xw2 tiles per 2KB bank) so the
    PSUM->SBUF copies are few and wide; the agg phase of a completed quad
    is interleaved one tile later to hide the copy latency.
  - Elementwise runs on [128, 896] bf16 operands (DVE 2x mode where the
    access pattern allows), reductions on Pool, exp/ln chains on ACT.
  - 1/den and rsqrt go through exp/ln so one ACT table load suffices.
"""

import numpy as np
import ml_dtypes
from contextlib import ExitStack
from dataclasses import dataclass

import concourse.bass as bass
import concourse.tile as tile
from concourse import bacc, mybir
from concourse.bass_utils import run_bass_kernel_spmd

# The stock ACT-table chooser greedily picks the first set containing each
# function, which for {Exp, Ln, Copy, Square} can alternate between two sets
# and reload the table per use (~1.3us each).  Restrict it to the one set
# that contains all of them so a single load suffices.
_ACT_KEEP = "natural_log_exp_and_others"
if not getattr(bacc, "_act_tables_patched", False):
    _orig_get_tables = bacc.get_activation_tables

    def _patched_get_tables(arch):
        t = _orig_get_tables(arch)
        if _ACT_KEEP in t:
            t = {k: (v if k == _ACT_KEEP else set()) for k, v in t.items()}
        return t

    bacc.get_activation_tables = _patched_get_tables
    bacc._act_tables_patched = True

BF16 = ml_dtypes.bfloat16
EPS = 1e-4
LN_EPS = 1e-5
P = 128
D = 128


@dataclass
class Cfg:
    N: int            # total nodes
    E: int            # total edges
    NC: int           # cores
    S: int = 0        # edge slot groups per tile (set by prep)
    SG: int = 14      # tiles per super-group (pipelining granule)
    ln_trivial: bool = False

    @property
    def NLOC(self):
        return self.N // self.NC

    @property
    def T(self):
        return (self.NLOC + P - 1) // P

    @property
    def TLP(self):
        return self.T * P

    @property
    def NSG(self):
        assert self.T % self.SG == 0
        return self.T // self.SG

    @property
    def SP1(self):    # slot groups incl. the self group
        return self.S + 1


def prep(x, edge_index, degree, W_fc, b_fc, W_rate, W_rob, b_rob, ln_gamma, ln_beta,
         cfg: Cfg):
    """Host-side preprocessing: shard + stage per-edge source rows by dst tile."""
    N, NC = cfg.N, cfg.NC
    NLOC, T, TLP = cfg.NLOC, cfg.T, cfg.TLP

    x = np.asarray(x, np.float32)
    edge_index = np.asarray(edge_index, np.int64)
    degree = np.asarray(degree)
    row, col = edge_index[0], edge_index[1]

    xbf = x.astype(BF16)
    xbf_ext = np.concatenate([xbf, np.zeros((1, D), BF16)], axis=0)  # pad row

    wfc = np.ascontiguousarray(W_fc, dtype=np.float32).astype(BF16)
    w2 = np.zeros((P, 2 * D), BF16)
    w2[:, 0:D] = np.asarray(W_rate, np.float32).astype(BF16)
    w2[:, D:2 * D] = np.asarray(W_rob, np.float32).astype(BF16)
    brob = np.zeros((1, 2 * D), np.float32)
    brob[0, D:2 * D] = np.asarray(b_rob, np.float32)
    bfcrow = np.asarray(b_fc, np.float32).reshape(1, D).copy()
    onesrow = np.ones((1, D), np.float32)
    ident = np.eye(P, dtype=BF16)

    cfg.ln_trivial = bool(np.all(np.asarray(ln_gamma) == 1.0)
                          and np.all(np.asarray(ln_beta) == 0.0))
    lnab = np.zeros((P, 2 * D), np.float32)
    lnab[:, :D] = np.asarray(ln_gamma, np.float32)[None, :]
    lnab[:, D:] = np.asarray(ln_beta, np.float32)[None, :]

    core_of = row // NLOC

    # pass 1: per-core per-tile edge counts fix the global S (slot groups/tile)
    percore = []
    S = 1
    for r in range(NC):
        m = core_of == r
        rl = row[m] - r * NLOC
        ce = col[m]
        cnt = np.bincount(rl, minlength=TLP)
        cnt_t = np.bincount(rl // P, minlength=T)
        S = max(S, -(-int(cnt_t.max()) // P))
        percore.append((rl, ce, cnt, cnt_t))
    cfg.S = S
    SP1 = S + 1
    SPT = SP1 * P  # slots per tile incl. self group

    in_maps = []
    for r in range(NC):
        rl, ce, cnt, cnt_t = percore[r]
        # order edges by (tile, col) -> fill each tile's slots densely; the
        # col ordering gives the staging gather some source locality
        order = np.lexsort((ce, rl // P))
        rl_s, ce_s = rl[order], ce[order]
        t_s = rl_s // P
        run_start = np.zeros(T + 1, np.int64)
        np.cumsum(cnt_t, out=run_start[1:])
        pos = np.arange(len(rl_s)) - run_start[t_s]     # slot within tile
        slot = t_s * SPT + pos                           # global slot id

        # staged source rows, SBUF layout [128, T*SP1*128] bf16:
        # partition = slot % 128, free = (slot//128)*128 + d.
        # group S of each tile holds the tile's own 128 rows (self term).
        src = np.full(T * SPT, N, np.int64)              # pads -> zero row
        src[slot] = ce_s
        own = r * NLOC + np.arange(TLP)
        own[NLOC:] = N
        src.reshape(T, SP1, P)[:, S, :] = own.reshape(T, P)
        xe_sb = np.ascontiguousarray(
            xbf_ext[src].reshape(T * SP1, P, D).transpose(1, 0, 2)
        ).reshape(P, T * SP1 * D)

        # rowsr: dst-in-tile per edge slot, -1 for pads -> zero sel column
        rowsr = np.full((P, T * S), -1.0, BF16)
        rowsr[pos % P, t_s * S + pos // P] = (rl_s % P).astype(BF16)

        iotab = np.broadcast_to(
            np.arange(P, dtype=BF16)[None, None, :], (P, S, P)
        ).reshape(P, S * P).copy()

        cnt2 = (2.0 * cnt).astype(np.float32)[None, :]
        cntb = cnt.astype(np.float32).reshape(T, P).T.copy()
        degl = np.zeros(TLP, np.float32)
        degl[:NLOC] = degree[r * NLOC:(r + 1) * NLOC].astype(np.float32)
        degf = degl.reshape(T, P).T.copy()

        xTloc = np.zeros((P, TLP), BF16)
        xTloc[:, :NLOC] = xbf[r * NLOC:(r + 1) * NLOC].T

        in_maps.append({
            "xe": xe_sb, "xT": xTloc,
            "wfc": wfc, "w2": w2, "brob": brob, "bfcrow": bfcrow,
            "onesrow": onesrow, "ident": ident, "lnab": lnab,
            "iotab": iotab, "rowsr": rowsr,
            "cnt2": cnt2, "cntb": cntb, "degf": degf,
        })
    return in_maps


def build(cfg: Cfg):
    """Build the SPMD Bass program (identical on every core)."""
    NC, T, TLP = cfg.NC, cfg.T, cfg.TLP
    S, SP1, SG, NSG = cfg.S, cfg.SP1, cfg.SG, cfg.NSG
    SPT = SP1 * P
    bf = mybir.dt.bfloat16
    f32 = mybir.dt.float32
    AO = mybir.AluOpType
    AF = mybir.ActivationFunctionType

    nc = bacc.Bacc("TRN2", target_bir_lowering=False, debug=False, num_devices=NC)
    for val in (LN_EPS, 1.0 + EPS):
        cs = nc.alloc_sbuf_tensor(f"const-float32-{val}", [P, 1], f32)
        nc.gpsimd.memset(cs.ap(), val)
        nc.const_aps.aps[(f32, val)] = cs.ap()
    nc.all_engine_barrier()

    d_xe = nc.dram_tensor("xe", [P, T * SPT], bf, kind="ExternalInput").ap()
    d_xT = nc.dram_tensor("xT", [P, TLP], bf, kind="ExternalInput").ap()
    d_wfc = nc.dram_tensor("wfc", [P, D], bf, kind="ExternalInput").ap()
    d_w2 = nc.dram_tensor("w2", [P, 2 * D], bf, kind="ExternalInput").ap()
    d_brob = nc.dram_tensor("brob", [1, 2 * D], f32, kind="ExternalInput").ap()
    d_bfc = nc.dram_tensor("bfcrow", [1, D], f32, kind="ExternalInput").ap()
    d_ones = nc.dram_tensor("onesrow", [1, D], f32, kind="ExternalInput").ap()
    d_ident = nc.dram_tensor("ident", [P, P], bf, kind="ExternalInput").ap()
    d_lnab = nc.dram_tensor("lnab", [P, 2 * D], f32, kind="ExternalInput").ap()
    d_iota = nc.dram_tensor("iotab", [P, S * P], bf, kind="ExternalInput").ap()
    d_rowsr = nc.dram_tensor("rowsr", [P, T * S], bf, kind="ExternalInput").ap()
    d_cnt2 = nc.dram_tensor("cnt2", [1, TLP], f32, kind="ExternalInput").ap()
    d_cntb = nc.dram_tensor("cntb", [P, T], f32, kind="ExternalInput").ap()
    d_degf = nc.dram_tensor("degf", [P, T], f32, kind="ExternalInput").ap()
    d_y = nc.dram_tensor("y", [TLP, D], bf, kind="ExternalOutput").ap()

    with tile.TileContext(nc) as tc, ExitStack() as ctx:
        from concourse import library_config
        nc.gpsimd.load_library(library_config.standard)
        consts = ctx.enter_context(tc.tile_pool(name="consts", bufs=1))
        wfc = consts.tile([P, D], bf)
        nc.sync.dma_start(wfc[:], d_wfc[:])
        w2 = consts.tile([P, 2 * D], bf)
        nc.sync.dma_start(w2[:], d_w2[:])
        xlocT = consts.tile([P, TLP], bf)
        nc.sync.dma_start(xlocT[:], d_xT[:])
        brob = consts.tile([1, 2 * D], f32)
        nc.sync.dma_start(brob[:], d_brob[:])
        bfcrow = consts.tile([1, D], f32)
        nc.sync.dma_start(bfcrow[:], d_bfc[:])
        onesr = consts.tile([1, D], f32)
        nc.sync.dma_start(onesr[:], d_ones[:])
        ident = consts.tile([P, P], bf)
        nc.sync.dma_start(ident[:], d_ident[:])
        iota = consts.tile([P, S * P], bf)
        nc.sync.dma_start(iota[:], d_iota[:])
        rowsr = consts.tile([P, T * S], bf)
        nc.sync.dma_start(rowsr[:], d_rowsr[:])
        cnt2 = consts.tile([1, TLP], f32)
        nc.sync.dma_start(cnt2[:], d_cnt2[:])
        cntb = consts.tile([P, T], f32)
        nc.sync.dma_start(cntb[:], d_cntb[:])
        degf = consts.tile([P, T], f32)
        nc.sync.dma_start(degf[:], d_degf[:])
        lnab = None
        if not cfg.ln_trivial:
            lnab = consts.tile([P, 2 * D], f32)
            nc.sync.dma_start(lnab[:], d_lnab[:])

        xep = ctx.enter_context(tc.tile_pool(name="xep", bufs=2))
        selp = ctx.enter_context(tc.tile_pool(name="selp", bufs=5))
        xw2ps = ctx.enter_context(tc.tile_pool(name="xw2ps", bufs=3, space="PSUM"))
        sxps = ctx.enter_context(tc.tile_pool(name="sxps", bufs=2, space="PSUM"))
        aggps = ctx.enter_context(tc.tile_pool(name="aggps", bufs=2, space="PSUM"))
        sxtp = ctx.enter_context(tc.tile_pool(name="sxtp", bufs=3))
        sgp = ctx.enter_context(tc.tile_pool(name="sgp", bufs=2))
        ysgp = ctx.enter_context(tc.tile_pool(name="ysgp", bufs=1))
        stp = ctx.enter_context(tc.tile_pool(name="stp", bufs=2))

        def emit_compute(sg):
            """Pass 1+2 for super-group sg: GEMMs, sel, x-space segment sums,
            agg.  Returns the context needed by the (deferred) elementwise."""
            t0 = sg * SG
            xe = xep.tile([P, SG * SPT], bf, tag="xe", name="xe")
            nc.sync.dma_start(xe[:], d_xe[:, t0 * SPT:(t0 + SG) * SPT])

            rate_sg = sgp.tile([P, SG, D], bf, tag="rate", name="rate")
            gam_sg = sgp.tile([P, SG, D], bf, tag="gam", name="gam")
            agg_sg = sgp.tile([P, SG, D], bf, tag="agg", name="agg")

            quads = []       # (sx psum bank, quad width, start tile)
            pending = []     # completed quads awaiting their agg phase

            def flush_quad():
                sxb, qw, q0 = pending.pop(0)
                sxT = sxtp.tile([P, 4, D], bf, tag="sxT", name="sxT")
                nc.scalar.copy(sxT[:, 0:qw, :], sxb[:, 0:qw, :])
                aggb = aggps.tile([P, 4, D], f32, space="PSUM", tag="aggb",
                                  name="aggb")
                for i in range(qw):
                    t = t0 + q0 + i
                    nc.tensor.matmul(out=aggb[:, i, :], lhsT=sxT[:, i, :],
                                     rhs=wfc[:], start=True, stop=False)
                    nc.tensor.matmul(out=aggb[:, i, :],
                                     lhsT=cnt2[0:1, t * P:(t + 1) * P],
                                     rhs=bfcrow[0:1, :], start=False, stop=True)
                nc.scalar.copy(agg_sg[:, q0:q0 + qw, :], aggb[:, 0:qw, :])

            xw2b = None
            sxb = None
            for tl in range(SG):
                t = t0 + tl
                pi = tl % 2
                if pi == 0:
                    pw = min(2, SG - tl)
                    xw2b = xw2ps.tile([P, 2, 2 * D], f32, space="PSUM",
                                      tag="xw2", name="xw2")
                nc.tensor.matmul(out=xw2b[:, pi, :],
                                 lhsT=xlocT[:, t * P:(t + 1) * P],
                                 rhs=w2[:], start=True, stop=False)
                nc.tensor.matmul(out=xw2b[:, pi, :], lhsT=onesr[0:1, :],
                                 rhs=brob[0:1, :], start=False, stop=True)
                sel = selp.tile([P, SPT], bf, tag="sel", name="sel")
                rb = rowsr[:, t * S:(t + 1) * S][:, :, None] \
                    .to_broadcast([P, S, P])
                nc.vector.tensor_tensor(
                    out=sel[:, 0:S * P].rearrange("p (s m) -> p s m", s=S),
                    in0=rb, in1=iota.rearrange("p (s m) -> p s m", s=S),
                    op=AO.is_equal)
                nc.gpsimd.tensor_tensor(
                    out=sel[:, S * P:SP1 * P], in0=ident[:],
                    in1=cntb[:, t:t + 1].to_broadcast([P, P]), op=AO.mult)
                qi = tl % 4
                if qi == 0:
                    qw = min(4, SG - tl)
                    sxb = sxps.tile([P, 4, D], f32, space="PSUM", tag="sx",
                                    name="sx")
                    quads.append((sxb, qw, tl))
                for s in range(SP1):
                    g0 = (tl * SP1 + s) * D
                    nc.tensor.matmul(out=sxb[:, qi, :], lhsT=xe[:, g0:g0 + D],
                                     rhs=sel[:, s * P:(s + 1) * P],
                                     start=(s == 0), stop=(s == SP1 - 1))
                if tl == quads[-1][2] + quads[-1][1] - 1:
                    pending.append(quads[-1])
                # delay each quad's agg phase by one tile to hide the
                # PSUM->SBUF copy latency from the PE
                if pending and tl >= pending[0][2] + pending[0][1]:
                    flush_quad()
                if pi + 1 == pw:
                    # rate/gamma for the completed pair
                    b0 = tl - pi
                    nc.scalar.activation(out=rate_sg[:, b0:b0 + pw, :],
                                         in_=xw2b[:, 0:pw, 0:D], func=AF.Exp)
                    nc.scalar.copy(gam_sg[:, b0:b0 + pw, :],
                                   xw2b[:, 0:pw, D:2 * D])
            while pending:
                flush_quad()
            nc.scalar.activation(out=rate_sg[:], in_=rate_sg[:], func=AF.Ln,
                                 bias=1.0)
            return dict(t0=t0, rate=rate_sg, gam=gam_sg, agg=agg_sg)

        def emit_eltwise(cx):
            """Pass 3 for a super-group: batched elementwise + LayerNorm."""
            t0 = cx["t0"]
            rate_sg, gam_sg, agg_sg = cx["rate"], cx["gam"], cx["agg"]
            num_sg = sgp.tile([P, SG, D], bf, tag="num", name="num")
            y0_sg = sgp.tile([P, SG, D], bf, tag="y0", name="y0")
            yt = ysgp.tile([P, SG, D], bf, tag="yt", name="yt")
            st = stp.tile([P, 6 * SG], f32, tag="st", name="st")
            stb = stp.tile([P, 2 * SG], bf, tag="stb", name="stb")
            s1 = st[:, 0 * SG:1 * SG]
            s2 = st[:, 1 * SG:2 * SG]
            mean = st[:, 2 * SG:3 * SG]
            rstd = st[:, 3 * SG:4 * SG]
            msq = st[:, 4 * SG:5 * SG]
            var = st[:, 5 * SG:6 * SG]
            meanb_src = stb[:, 0:SG]
            rstdb_src = stb[:, SG:2 * SG]

            degb = degf[:, t0:t0 + SG][:, :, None].to_broadcast([P, SG, D])
            nc.vector.scalar_tensor_tensor(
                out=num_sg[:], in0=rate_sg[:], scalar=EPS, in1=agg_sg[:],
                op0=AO.add, op1=AO.mult)
            nc.vector.tensor_add(out=num_sg[:], in0=num_sg[:], in1=gam_sg[:])
            # gamma has been consumed: reuse its buffer for the 1/den chain
            invd_sg = gam_sg
            nc.vector.scalar_tensor_tensor(
                out=invd_sg[:], in0=rate_sg[:], scalar=EPS, in1=degb,
                op0=AO.add, op1=AO.mult)
            nc.scalar.activation(out=invd_sg[:], in_=invd_sg[:], func=AF.Ln,
                                 bias=1.0 + EPS)
            nc.scalar.activation(out=invd_sg[:], in_=invd_sg[:], func=AF.Exp,
                                 scale=-1.0)
            nc.vector.tensor_mul(out=y0_sg[:], in0=num_sg[:], in1=invd_sg[:])
            # LayerNorm stats per (node, tile)
            nc.vector.tensor_reduce(out=s1, in_=y0_sg[:],
                                    axis=mybir.AxisListType.X, op=AO.add)
            # reuse num_sg as the square scratch
            nc.vector.tensor_mul(out=num_sg[:], in0=y0_sg[:], in1=y0_sg[:])
            nc.vector.tensor_reduce(out=s2, in_=num_sg[:],
                                    axis=mybir.AxisListType.X, op=AO.add)
            nc.vector.tensor_scalar_mul(out=mean, in0=s1, scalar1=1.0 / D)
            nc.vector.tensor_scalar_mul(out=msq, in0=s2, scalar1=1.0 / D)
            nc.vector.tensor_tensor(out=var, in0=mean, in1=mean, op=AO.mult)
            nc.vector.tensor_sub(out=var, in0=msq, in1=var)
            nc.scalar.activation(out=var, in_=var, func=AF.Ln, bias=LN_EPS)
            nc.scalar.activation(out=rstd, in_=var, func=AF.Exp, scale=-0.5)
            nc.scalar.copy(stb[:], st[:, 2 * SG:4 * SG])
            meanb = meanb_src[:, :, None].to_broadcast([P, SG, D])
            rstdb = rstdb_src[:, :, None].to_broadcast([P, SG, D])
            nc.vector.tensor_sub(out=y0_sg[:], in0=y0_sg[:], in1=meanb)
            if lnab is None:
                nc.vector.tensor_mul(out=yt[:], in0=y0_sg[:], in1=rstdb)
            else:
                nc.vector.tensor_mul(out=y0_sg[:], in0=y0_sg[:], in1=rstdb)
                lg = lnab[:, 0:D][:, None, :].to_broadcast([P, SG, D])
                lb = lnab[:, D:2 * D][:, None, :].to_broadcast([P, SG, D])
                nc.vector.tensor_mul(out=y0_sg[:], in0=y0_sg[:], in1=lg)
                nc.vector.tensor_add(out=yt[:], in0=y0_sg[:], in1=lb)
            dst = d_y[t0 * P:(t0 + SG) * P, :].rearrange("(t p) d -> p t d",
                                                         p=P)
            nc.sync.dma_start(dst, yt[:])

        prev = None
        for sg in range(NSG):
            cx = emit_compute(sg)
            if prev is not None:
                emit_eltwise(prev)
            prev = cx
        emit_eltwise(prev)

    nc.compile()
    return nc


def run(inputs, cfg: Cfg, core_ids=None):
    in_maps = prep(**inputs, cfg=cfg)
    nc = build(cfg)
    res = run_bass_kernel_spmd(nc, in_maps, core_ids=core_ids or list(range(cfg.NC)))
    ys = [res.results[r]["y"][:cfg.NLOC] for r in range(cfg.NC)]
    return np.concatenate(ys, axis=0).astype(np.float32)


def kernel(**inputs):
    cfg = Cfg(N=100_000, E=800_000, NC=8)
    return run(inputs, cfg)
